# revision 6
# baseline (speedup 1.0000x reference)
"""L1-distance attention kernel for Trainium2 (8 NeuronCores, SPMD).

Problem: q, k: [B=2, T=512, H=8, D=64] fp32
         out[b,s,t,h] = -sum_d |q[b,s,h,d] - k[b,t,h,d]| / sqrt(D)

Sharding: 16 (b,h) pairs across 8 cores, 2 pairs per core, stacked in the
SBUF partition dim (pair0 -> partitions 0:64 holding d, pair1 -> 64:128).

Math: |q-k| = (q+k) - 2*min(q,k), so with Q_s = sum_d q[d,s], K_t = sum_d k[d,t]:
  -scale*sum_d|q-k| = 2*scale*sum_d min(k_t, q_s) - scale*K_t - scale*Q_s

All tensor data is bf16 (inputs rounded on host; identity above is exact in
bf16-value space, so only the input representation error ~2^-9 remains, well
under the 2e-2 gate). bf16 unlocks DVE 4x mode (min: 194ns vs 327 fp32) and
1 cycle/row matmuls (213ns vs 853 fp32).

Per core, per query s (512 total):
  - DVE tensor_scalar_min(k_bf16, q_fp32[:,s]) -> [128, 512] bf16 tile
    (scalar operand must be fp32; result still equals min(kb, qb) exactly
    after the bf16 output round).
  - TensorE: one bf16 matmul per query with a +2*scale selector weight
    [128, 32] routing each (pair, query) sum to its own PSUM row; 16 queries
    accumulate per 32-row block (col-tiled), preceded by one full-width
    -scale*K_t correction matmul per 64-query group.
  - After 64 queries a [128, 512] PSUM tile is done -> ScalarE Identity copy
    folds the per-row -scale*Q_s bias -> SBUF -> DMA out.

Host builds selector weights / Q-sum biases and unscrambles output rows.
"""

import os

import numpy as np
import ml_dtypes

os.environ.setdefault("MYCRO_LOCAL_CACHE", "1")

B, T, H, D = 2, 512, 8, 64
NCORES = 8
NGROUPS = 8  # query groups of 64 -> one PSUM tile each
SCALE = 1.0 / float(np.sqrt(np.float64(D)))  # 0.125

_cached = {}


def _build_module(reps=1):
    from concourse import bacc, tile
    import concourse.mybir as mybir

    f32 = mybir.dt.float32
    bf16 = mybir.dt.bfloat16
    nc = bacc.Bacc(
        "TRN2",
        target_bir_lowering=False,
        debug=False,
        enable_asserts=False,
        num_devices=1,
    )
    q_dram = nc.dram_tensor("q", [128, T], f32, kind="ExternalInput")
    k_dram = nc.dram_tensor("k", [128, T], bf16, kind="ExternalInput")
    w_dram = nc.dram_tensor("w", [128, 4, 16, 32], bf16, kind="ExternalInput")
    wk_dram = nc.dram_tensor("wk", [128, 128], bf16, kind="ExternalInput")
    qs_dram = nc.dram_tensor("qs", [128, NGROUPS], f32, kind="ExternalInput")
    out_dram = nc.dram_tensor("out", [NGROUPS, 128, T], f32, kind="ExternalOutput")

    with tile.TileContext(nc) as tc:
        with (
            tc.tile_pool(name="const", bufs=1) as cpool,
            tc.tile_pool(name="ad", bufs=8) as adpool,
            tc.tile_pool(name="osb", bufs=3) as opool,
            tc.tile_pool(name="psum", bufs=8, space="PSUM") as ppool,
        ):
            q_sb = cpool.tile([128, T], f32, tag="q")
            k_sb = cpool.tile([128, T], bf16, tag="k")
            w_sb = cpool.tile([128, 4, 16, 32], bf16, tag="w")
            wk_sb = cpool.tile([128, 128], bf16, tag="wk")
            qs_sb = cpool.tile([128, NGROUPS], f32, tag="qs")
            nc.sync.dma_start(q_sb[:], q_dram[:])
            nc.sync.dma_start(k_sb[:], k_dram[:])
            nc.sync.dma_start(w_sb[:], w_dram[:])
            nc.sync.dma_start(wk_sb[:], wk_dram[:])
            nc.sync.dma_start(qs_sb[:], qs_dram[:])

            for g in range(NGROUPS * reps):
                g = g % NGROUPS
                psum_t = ppool.tile([128, T], f32, tag="acc")
                # -scale * K_t correction for all 128 rows at once
                nc.tensor.matmul(
                    psum_t[:],
                    wk_sb[:],
                    k_sb[:],
                    start=True,
                    stop=False,
                )
                for c in range(4):
                    for jj in range(16):
                        s = 64 * g + 16 * c + jj
                        ad = adpool.tile([128, T], bf16, tag="ad")
                        nc.vector.tensor_scalar_min(
                            ad[:], k_sb[:], q_sb[:, s : s + 1]
                        )
                        nc.tensor.matmul(
                            psum_t[32 * c : 32 * c + 32, :],
                            w_sb[:, c, jj, :],
                            ad[:],
                            start=False,
                            stop=(jj == 15),
                            tile_position=(0, 32 * c),
                        )
                ob = opool.tile([128, T], f32, tag="ob")
                # copy + per-row bias (-scale*Q_s) on ScalarE
                nc.scalar.activation(
                    ob[:],
                    psum_t[:],
                    mybir.ActivationFunctionType.Identity,
                    bias=qs_sb[:, g : g + 1],
                    scale=1.0,
                )
                nc.sync.dma_start(out_dram[g], ob[:])

    nc.compile()
    return nc


def _host_weights():
    # Selector: row r = 32c + 2jj + p of a group gets 2*scale * pair-p sum of
    # query s = 64g + 16c + jj. Exact in bf16 (0.25).
    w = np.zeros((128, 4, 16, 32), np.float32)
    for c in range(4):
        for jj in range(16):
            w[0:64, c, jj, 2 * jj] = 2.0 * SCALE
            w[64:128, c, jj, 2 * jj + 1] = 2.0 * SCALE
    wk = np.zeros((128, 128), np.float32)
    wk[0:64, 0::2] = -SCALE
    wk[64:128, 1::2] = -SCALE
    return w.astype(ml_dtypes.bfloat16), wk.astype(ml_dtypes.bfloat16)


def _host_qsum(qb):
    """qb: [128, T] per-core stacked q^T in bf16. Returns qs [128, NGROUPS]
    fp32: row r = 32c + 2jj + p of group g gets -scale*sum_d qb[pair p, d, s]
    with s = 64g + 16c + jj."""
    qsum = qb.astype(np.float64).reshape(2, 64, T).sum(axis=1)  # [pair, s]
    qs = np.empty((128, NGROUPS), np.float64)
    for g in range(NGROUPS):
        for c in range(4):
            for jj in range(16):
                s = 64 * g + 16 * c + jj
                for p in range(2):
                    qs[32 * c + 2 * jj + p, g] = -SCALE * qsum[p, s]
    return qs.astype(np.float32)


def get_module(reps=1):
    key = ("nc", reps)
    nc = _cached.get(key)
    if nc is None:
        nc = _build_module(reps)
        _cached[key] = nc
    return nc


def make_in_maps(q, k):
    """Shard full [B,T,H,D] q/k into 8 per-core input maps."""
    q = np.asarray(q, dtype=np.float32)
    k = np.asarray(k, dtype=np.float32)
    # [B, T, H, D] -> [B, H, D, T] -> [B*H, D, T]
    qt = np.ascontiguousarray(q.transpose(0, 2, 3, 1)).reshape(B * H, D, T)
    kt = np.ascontiguousarray(k.transpose(0, 2, 3, 1)).reshape(B * H, D, T)
    w, wk = _host_weights()
    in_maps = []
    for c in range(NCORES):
        qc = np.ascontiguousarray(qt[2 * c : 2 * c + 2].reshape(128, T))
        kc = np.ascontiguousarray(kt[2 * c : 2 * c + 2].reshape(128, T))
        qb = qc.astype(ml_dtypes.bfloat16)
        kb = kc.astype(ml_dtypes.bfloat16)
        in_maps.append(
            {
                # fp32 scalar source for tensor_scalar_min, pre-rounded so
                # min(kb, q) == min(kb, qb) exactly
                "q": qb.astype(np.float32),
                "k": kb,
                "w": w,
                "wk": wk,
                "qs": _host_qsum(qb),
            }
        )
    return in_maps


def assemble_output(core_outs):
    """core_outs: list of 8 arrays [NGROUPS, 128, T] -> full [B, T, T, H]."""
    outf = np.empty((B, T, T, H), np.float32)
    for c in range(NCORES):
        o = np.asarray(core_outs[c]).reshape(NGROUPS, 4, 16, 2, T)
        # row r = 32c + 2jj + p in group g  ->  query s = 64g + 16c + jj
        o = o.transpose(3, 0, 1, 2, 4).reshape(2, T, T)
        for p in range(2):
            pg = 2 * c + p
            b, h = divmod(pg, H)
            outf[b, :, :, h] = o[p]
    return outf


def kernel(q, k):
    from concourse.bass_utils import run_bass_kernel_spmd

    nc = get_module()
    in_maps = make_in_maps(q, k)
    res = run_bass_kernel_spmd(
        nc,
        in_maps,
        core_ids=list(range(NCORES)),
        trace=os.environ.get("BASS_L1_TRACE", "0") == "1",
    )
    _cached["last_results"] = res
    return assemble_output([r["out"] for r in res.results])


# revision 24
# speedup vs baseline: 1.1672x; 1.1672x over previous
"""L1-distance attention kernel for Trainium2 (8 NeuronCores, SPMD).

Problem: q, k: [B=2, T=512, H=8, D=64] fp32
         out[b,s,t,h] = -sum_d |q[b,s,h,d] - k[b,t,h,d]| / sqrt(D)

Sharding: 16 (b,h) pairs across 8 cores, 2 pairs per core, stacked in the
SBUF partition dim with layout (dhalf, pair, d32): partition
p = dhalf*64 + pair*32 + (d%32), dhalf = d//32. This makes the d-half fold
(below) a contiguous partition-range add.

Math: |q-k| = (q+k) - 2*min(q,k) and min(q,k) = q - relu(q-k), so with
Q_s = sum_d q[d,s], K_t = sum_d k[d,t]:
  -scale*sum_d|q-k| = 2*scale*sum_d min(k_t, q_s) - scale*K_t - scale*Q_s
                    = -2*scale*sum_d relu(q_s-k_t) - scale*K_t + scale*Q_s

All tensor data is bf16 (inputs rounded on host; the identities are exact
in bf16-value space, so only input representation error ~2^-9 and the
fold's bf16 rounding remain, far under the 2e-2 gate).

Per core, per 64-query group -> one [128, 512] fp32 PSUM tile (row
r = 32c + 2jj + p for block c, slot jj, pair p):
  - one full-width -scale*K_t matmul seeds the accumulation (start=True),
  - slots jj=0..11 (DVE/min identity): tensor_scalar_min (194ns) ->
    bf16 [128,512] tile -> one [128,32]x[128,512] bf16 selector matmul
    (213ns, weight +2*scale),
  - slots jj=12..15 (ACT/relu identity) are FOLDED in pairs (12,13) and
    (14,15): host prebuilds duplicated-row tiles kf_lo=[k_lo;k_lo],
    kf_hi=[k_hi;k_hi] and interleaved scalar columns qf_lo/qf_hi, so ONE
    ScalarE relu op produces both queries' lo-half tiles (and one more the
    hi-halves), then ONE Pool tensor_tensor add folds d-halves for the
    whole pair -> [128,512] moving tile -> ONE matmul (weight -2*scale)
    covers 2 queries. PE does 60 matmuls/group instead of 64; the fold
    work lands on the otherwise-idle Scalar/Pool engines.
  - ScalarE Identity copy folds the per-row +-scale*Q_s bias -> SBUF -> DMA.

The last group splits its final 32 rows into the warmup PSUM tile so the
main 96-row copy+DMA overlaps the final block's matmuls (shorter tail).

Host builds selector weights / Q-sum biases and unscrambles output rows.
"""

import os

import numpy as np
import ml_dtypes

os.environ.setdefault("MYCRO_LOCAL_CACHE", "1")

B, T, H, D = 2, 512, 8, 64
NCORES = 8
NGROUPS = 8  # query groups of 64 -> one PSUM tile each
SCALE = 1.0 / float(np.sqrt(np.float64(D)))  # 0.125
FOLD_PAIRS = ((12, 13), (14, 15))  # folded slot pairs per block
NF = NGROUPS * 4 * len(FOLD_PAIRS)  # fold-scalar columns

# partition permutation: PERM[newp] = old row (pair*64 + d)
PERM = np.empty(128, np.int64)
for _pair in range(2):
    for _d in range(64):
        PERM[(_d // 32) * 64 + _pair * 32 + (_d % 32)] = _pair * 64 + _d

_cached = {}


def _fold_col(g, c, pi):
    return (g * 4 + c) * len(FOLD_PAIRS) + pi


def _build_module(reps=1):
    from concourse import bacc, tile
    import concourse.mybir as mybir

    f32 = mybir.dt.float32
    bf16 = mybir.dt.bfloat16
    nc = bacc.Bacc(
        "TRN2",
        target_bir_lowering=False,
        debug=False,
        enable_asserts=False,
        num_devices=1,
    )
    q_dram = nc.dram_tensor("q", [128, T], f32, kind="ExternalInput")
    k_dram = nc.dram_tensor("k", [128, T], bf16, kind="ExternalInput")
    kf_lo_dram = nc.dram_tensor("kf_lo", [128, T], bf16, kind="ExternalInput")
    kf_hi_dram = nc.dram_tensor("kf_hi", [128, T], bf16, kind="ExternalInput")
    qf_lo_dram = nc.dram_tensor("qf_lo", [128, NF], f32, kind="ExternalInput")
    qf_hi_dram = nc.dram_tensor("qf_hi", [128, NF], f32, kind="ExternalInput")
    w_dram = nc.dram_tensor("w", [128, 4, 12, 32], bf16, kind="ExternalInput")
    w2_dram = nc.dram_tensor("w2", [128, 2, 32], bf16, kind="ExternalInput")
    wk_dram = nc.dram_tensor("wk", [128, 128], bf16, kind="ExternalInput")
    qs_dram = nc.dram_tensor("qs", [128, NGROUPS], f32, kind="ExternalInput")
    out_dram = nc.dram_tensor("out", [NGROUPS, 128, T], f32, kind="ExternalOutput")

    warmup = 10
    with tile.TileContext(nc) as tc:
        with (
            tc.tile_pool(name="const", bufs=1) as cpool,
            tc.tile_pool(name="ad", bufs=8) as adpool,
            tc.tile_pool(name="rl", bufs=6) as rlpool,
            tc.tile_pool(name="ft", bufs=4) as ftpool,
            tc.tile_pool(name="osb", bufs=3) as opool,
            tc.tile_pool(name="psum", bufs=4, space="PSUM") as ppool,
            tc.tile_pool(name="wpsum", bufs=1, space="PSUM") as wppool,
        ):
            q_sb = cpool.tile([128, T], f32, tag="q")
            k_sb = cpool.tile([128, T], bf16, tag="k")
            kf_lo = cpool.tile([128, T], bf16, tag="kflo")
            kf_hi = cpool.tile([128, T], bf16, tag="kfhi")
            qf_lo = cpool.tile([128, NF], f32, tag="qflo")
            qf_hi = cpool.tile([128, NF], f32, tag="qfhi")
            w_sb = cpool.tile([128, 4, 12, 32], bf16, tag="w")
            w2_sb = cpool.tile([128, 2, 32], bf16, tag="w2")
            wk_sb = cpool.tile([128, 128], bf16, tag="wk")
            qs_sb = cpool.tile([128, NGROUPS], f32, tag="qs")

            # PE warmup: ramp the Tensor engine to full pstate on junk data
            # while the input DMAs are in flight.
            wmv = cpool.tile([128, 128], bf16, tag="wmv")
            nc.gpsimd.memset(wmv[:], 0.0)
            wps = wppool.tile([128, T], f32, tag="wps")
            for _ in range(warmup):
                nc.tensor.matmul(
                    wps[0:32, 0:128], wmv[:, 0:32], wmv[:], start=True, stop=True
                )

            # Inputs spread across DGE queues (sync/scalar/gpsimd) so the
            # copies run in parallel, ordered by first use.
            nc.sync.dma_start(wk_sb[:], wk_dram[:])
            nc.sync.dma_start(k_sb[:], k_dram[:])
            nc.sync.dma_start(kf_lo[:], kf_lo_dram[:])
            nc.sync.dma_start(kf_hi[:], kf_hi_dram[:])
            nc.sync.dma_start(w2_sb[:], w2_dram[:])
            nc.scalar.dma_start(q_sb[:], q_dram[:])
            nc.scalar.dma_start(w_sb[:, 0:2], w_dram[:, 0:2])
            nc.scalar.dma_start(w_sb[:, 2:4], w_dram[:, 2:4])
            nc.scalar.dma_start(qf_lo[:], qf_lo_dram[:])
            nc.scalar.dma_start(qf_hi[:], qf_hi_dram[:])
            nc.gpsimd.dma_start(qs_sb[:], qs_dram[:])

            def emit_block(g, c, blk, blk_pos):
                """Producers + matmuls for block c of group g into psum blk."""
                # folded pairs first: ScalarE makes both queries' relu
                # halves, Pool folds d-halves for the whole pair at once
                fts = []
                for pi in range(len(FOLD_PAIRS)):
                    m = _fold_col(g, c, pi)
                    rl_lo = rlpool.tile([128, T], bf16, tag="rl")
                    nc.scalar.activation(
                        rl_lo[:],
                        kf_lo[:],
                        mybir.ActivationFunctionType.Relu,
                        bias=qf_lo[:, m : m + 1],
                        scale=-1.0,
                    )
                    rl_hi = rlpool.tile([128, T], bf16, tag="rl")
                    nc.scalar.activation(
                        rl_hi[:],
                        kf_hi[:],
                        mybir.ActivationFunctionType.Relu,
                        bias=qf_hi[:, m : m + 1],
                        scale=-1.0,
                    )
                    ft = ftpool.tile([128, T], bf16, tag="ft")
                    nc.gpsimd.tensor_tensor(
                        ft[:], rl_lo[:], rl_hi[:], mybir.AluOpType.add
                    )
                    fts.append(ft)
                for jj in range(12):
                    s = 64 * g + 16 * c + jj
                    ad = adpool.tile([128, T], bf16, tag="ad")
                    nc.vector.tensor_scalar_min(
                        ad[:], k_sb[:], q_sb[:, s : s + 1]
                    )
                    nc.tensor.matmul(
                        blk,
                        w_sb[:, c, jj, :],
                        ad[:],
                        start=False,
                        stop=False,
                        tile_position=blk_pos,
                    )
                for pi, ft in enumerate(fts):
                    nc.tensor.matmul(
                        blk,
                        w2_sb[:, pi, :],
                        ft[:],
                        start=False,
                        stop=(pi == len(fts) - 1),
                        tile_position=blk_pos,
                    )

            total = NGROUPS * reps
            for gi in range(total):
                g = gi % NGROUPS
                last = gi == total - 1
                psum_t = ppool.tile([128, T], f32, tag="acc")
                # -scale * K_t correction (all 128 rows; 96 on the last
                # group, whose final block lives in the warmup psum tile so
                # the main copy+DMA can overlap its matmuls)
                nc.tensor.matmul(
                    psum_t[0:96, :] if last else psum_t[:],
                    wk_sb[:, 0:96] if last else wk_sb[:],
                    k_sb[:],
                    start=True,
                    stop=False,
                )
                if last:
                    nc.tensor.matmul(
                        wps[0:32, :],
                        wk_sb[:, 96:128],
                        k_sb[:],
                        start=True,
                        stop=False,
                        tile_position=(0, 0),
                    )
                ob = opool.tile([128, T], f32, tag="ob")
                for c in range(4):
                    blk = (
                        wps[0:32, :]
                        if (last and c == 3)
                        else psum_t[32 * c : 32 * c + 32, :]
                    )
                    emit_block(
                        g, c, blk, (0, 0) if (last and c == 3) else (0, 32 * c)
                    )
                    if last and c == 2:
                        # blocks 0-2 final: copy + bias + DMA now, overlapping
                        # block 3's matmuls
                        nc.scalar.activation(
                            ob[0:96, :],
                            psum_t[0:96, :],
                            mybir.ActivationFunctionType.Identity,
                            bias=qs_sb[0:96, g : g + 1],
                            scale=1.0,
                        )
                        nc.sync.dma_start(out_dram[g, 0:96, :], ob[0:96, :])
                # copy + per-row bias (+-scale*Q_s) on ScalarE
                if last:
                    nc.scalar.activation(
                        ob[96:128, :],
                        wps[0:32, :],
                        mybir.ActivationFunctionType.Identity,
                        bias=qs_sb[96:128, g : g + 1],
                        scale=1.0,
                    )
                    nc.sync.dma_start(out_dram[g, 96:128, :], ob[96:128, :])
                else:
                    nc.scalar.activation(
                        ob[:],
                        psum_t[:],
                        mybir.ActivationFunctionType.Identity,
                        bias=qs_sb[:, g : g + 1],
                        scale=1.0,
                    )
                    nc.sync.dma_start(out_dram[g], ob[:])

    nc.compile()
    return nc


def _host_weights():
    # Unfolded (min-identity) selector: row r = 32c + 2jj + p gets +2*scale
    # on the pair-p partition rows of the (dhalf, pair, d32) layout.
    pair_rows = np.zeros((2, 128), bool)
    for p in range(2):
        pair_rows[p, p * 32 : (p + 1) * 32] = True
        pair_rows[p, 64 + p * 32 : 64 + (p + 1) * 32] = True
    w = np.zeros((128, 4, 12, 32), np.float32)
    for c in range(4):
        for jj in range(12):
            for p in range(2):
                w[pair_rows[p], c, jj, 2 * jj + p] = 2.0 * SCALE
    # Folded (relu-identity) selector: moving tile = [foldA (pair,d32) 64;
    # foldB 64] for pair (jjA, jjB); weight -2*scale.
    w2 = np.zeros((128, 2, 32), np.float32)
    for pi, (ja, jb) in enumerate(FOLD_PAIRS):
        w2[0:32, pi, 2 * ja] = -2.0 * SCALE
        w2[32:64, pi, 2 * ja + 1] = -2.0 * SCALE
        w2[64:96, pi, 2 * jb] = -2.0 * SCALE
        w2[96:128, pi, 2 * jb + 1] = -2.0 * SCALE
    # K_t correction: -scale on every (pair,d) row of matching pair
    wk = np.zeros((128, 128), np.float32)
    for p in range(2):
        wk[np.ix_(pair_rows[p], np.arange(p, 128, 2))] = -SCALE
    return (
        w.astype(ml_dtypes.bfloat16),
        w2.astype(ml_dtypes.bfloat16),
        wk.astype(ml_dtypes.bfloat16),
    )


def _host_qsum(qb):
    """qb: [128, T] per-core stacked q^T in bf16, (pair,d) layout (pre-PERM).
    Returns qs [128, NGROUPS] fp32: row r = 32c + 2jj + p of group g gets
    -+scale*sum_d qb[pair p, d, s] (+ for folded/relu slots jj>=12)."""
    qsum = qb.astype(np.float64).reshape(2, 64, T).sum(axis=1)  # [pair, s]
    folded = {jj for pr in FOLD_PAIRS for jj in pr}
    qs = np.empty((128, NGROUPS), np.float64)
    for g in range(NGROUPS):
        for c in range(4):
            for jj in range(16):
                s = 64 * g + 16 * c + jj
                sign = 1.0 if jj in folded else -1.0
                for p in range(2):
                    qs[32 * c + 2 * jj + p, g] = sign * SCALE * qsum[p, s]
    return qs.astype(np.float32)


def get_module(reps=1):
    key = ("nc", reps)
    nc = _cached.get(key)
    if nc is None:
        nc = _build_module(reps)
        _cached[key] = nc
    return nc


def make_in_maps(q, k):
    """Shard full [B,T,H,D] q/k into 8 per-core input maps."""
    q = np.asarray(q, dtype=np.float32)
    k = np.asarray(k, dtype=np.float32)
    # [B, T, H, D] -> [B, H, D, T] -> [B*H, D, T]
    qt = np.ascontiguousarray(q.transpose(0, 2, 3, 1)).reshape(B * H, D, T)
    kt = np.ascontiguousarray(k.transpose(0, 2, 3, 1)).reshape(B * H, D, T)
    w, w2, wk = _host_weights()
    in_maps = []
    for core in range(NCORES):
        qc = np.ascontiguousarray(qt[2 * core : 2 * core + 2].reshape(128, T))
        kc = np.ascontiguousarray(kt[2 * core : 2 * core + 2].reshape(128, T))
        qb = qc.astype(ml_dtypes.bfloat16)
        kb = kc.astype(ml_dtypes.bfloat16)
        # PERM layout views
        qp = qb.astype(np.float32)[PERM]  # fp32 scalar source, pre-rounded
        kp = kb[PERM]
        # duplicated-row fold tiles and interleaved fold scalars
        kf_lo = np.concatenate([kp[0:64], kp[0:64]])
        kf_hi = np.concatenate([kp[64:128], kp[64:128]])
        qf_lo = np.empty((128, NF), np.float32)
        qf_hi = np.empty((128, NF), np.float32)
        for g in range(NGROUPS):
            for c in range(4):
                for pi, (ja, jb) in enumerate(FOLD_PAIRS):
                    m = _fold_col(g, c, pi)
                    sa = 64 * g + 16 * c + ja
                    sb = 64 * g + 16 * c + jb
                    qf_lo[0:64, m] = qp[0:64, sa]
                    qf_lo[64:128, m] = qp[0:64, sb]
                    qf_hi[0:64, m] = qp[64:128, sa]
                    qf_hi[64:128, m] = qp[64:128, sb]
        in_maps.append(
            {
                "q": np.ascontiguousarray(qp),
                "k": np.ascontiguousarray(kp),
                "kf_lo": np.ascontiguousarray(kf_lo),
                "kf_hi": np.ascontiguousarray(kf_hi),
                "qf_lo": qf_lo,
                "qf_hi": qf_hi,
                "w": w,
                "w2": w2,
                "wk": wk,
                "qs": _host_qsum(qb),
            }
        )
    return in_maps


def assemble_output(core_outs):
    """core_outs: list of 8 arrays [NGROUPS, 128, T] -> full [B, T, T, H]."""
    outf = np.empty((B, T, T, H), np.float32)
    for core in range(NCORES):
        o = np.asarray(core_outs[core]).reshape(NGROUPS, 4, 16, 2, T)
        # row r = 32c + 2jj + p in group g  ->  query s = 64g + 16c + jj
        o = o.transpose(3, 0, 1, 2, 4).reshape(2, T, T)
        for p in range(2):
            pg = 2 * core + p
            b, h = divmod(pg, H)
            outf[b, :, :, h] = o[p]
    return outf


def kernel(q, k):
    from concourse.bass_utils import run_bass_kernel_spmd

    nc = get_module()
    in_maps = make_in_maps(q, k)
    res = run_bass_kernel_spmd(
        nc,
        in_maps,
        core_ids=list(range(NCORES)),
        trace=os.environ.get("BASS_L1_TRACE", "0") == "1",
    )
    _cached["last_results"] = res
    return assemble_output([r["out"] for r in res.results])


# revision 29
# speedup vs baseline: 1.2036x; 1.0312x over previous
"""L1-distance attention kernel for Trainium2 (8 NeuronCores, SPMD).

Problem: q, k: [B=2, T=512, H=8, D=64] fp32
         out[b,s,t,h] = -sum_d |q[b,s,h,d] - k[b,t,h,d]| / sqrt(D)

Sharding: 16 (b,h) pairs across 8 cores, 2 pairs per core, stacked in the
SBUF partition dim with layout (dhalf, pair, d32): partition
p = dhalf*64 + pair*32 + (d%32), dhalf = d//32. This makes the d-half fold
(below) a contiguous partition-range add.

Math: |q-k| = (q+k) - 2*min(q,k) and min(q,k) = q - relu(q-k), so with
Q_s = sum_d q[d,s], K_t = sum_d k[d,t]:
  -scale*sum_d|q-k| = 2*scale*sum_d min(k_t, q_s) - scale*K_t - scale*Q_s
                    = -2*scale*sum_d relu(q_s-k_t) - scale*K_t + scale*Q_s

All tensor data is bf16 (inputs rounded on host; the identities are exact
in bf16-value space, so only input representation error ~2^-9 and the
fold's bf16 rounding remain, far under the 2e-2 gate).

Per core, per 64-query group -> one [128, 512] fp32 PSUM tile (row
r = 32c + 2jj + p for block c, slot jj, pair p):
  - one full-width -scale*K_t matmul seeds the accumulation (start=True),
  - slots jj=0..11 (DVE/min identity): tensor_scalar_min (194ns) ->
    bf16 [128,512] tile -> one [128,32]x[128,512] bf16 selector matmul
    (213ns, weight +2*scale),
  - slots jj=12..15 (ACT/relu identity) are FOLDED in pairs (12,13) and
    (14,15): host prebuilds duplicated-row tiles kf_lo=[k_lo;k_lo],
    kf_hi=[k_hi;k_hi] and interleaved scalar columns qf_lo/qf_hi, so ONE
    ScalarE relu op produces both queries' lo-half tiles (and one more the
    hi-halves), then ONE Pool tensor_tensor add folds d-halves for the
    whole pair -> [128,512] moving tile -> ONE matmul (weight -2*scale)
    covers 2 queries. PE does 60 matmuls/group instead of 64; the fold
    work lands on the otherwise-idle Scalar/Pool engines.
  - ScalarE Identity copy folds the per-row +-scale*Q_s bias -> SBUF -> DMA.

The last group splits its final 32 rows into the warmup PSUM tile so the
main 96-row copy+DMA overlaps the final block's matmuls (shorter tail).

Host builds selector weights / Q-sum biases and unscrambles output rows.
"""

import os

import numpy as np
import ml_dtypes

os.environ.setdefault("MYCRO_LOCAL_CACHE", "1")

B, T, H, D = 2, 512, 8, 64
NCORES = 8
NGROUPS = 8  # query groups of 64 -> one PSUM tile each
SCALE = 1.0 / float(np.sqrt(np.float64(D)))  # 0.125
FOLD_PAIRS = ((12, 13), (14, 15))  # ACT-produced folded pairs, every block
XTRA_PAIR = (10, 11)  # DVE-produced pair, folded on odd blocks only
NPI = 3
NF = NGROUPS * 4 * NPI  # fold-scalar columns

# partition permutation: PERM[newp] = old row (pair*64 + d)
PERM = np.empty(128, np.int64)
for _pair in range(2):
    for _d in range(64):
        PERM[(_d // 32) * 64 + _pair * 32 + (_d % 32)] = _pair * 64 + _d

_cached = {}


def _fold_col(g, c, pi):
    return (g * 4 + c) * NPI + pi


def _build_module(reps=1):
    from concourse import bacc, tile
    import concourse.mybir as mybir

    f32 = mybir.dt.float32
    bf16 = mybir.dt.bfloat16
    nc = bacc.Bacc(
        "TRN2",
        target_bir_lowering=False,
        debug=False,
        enable_asserts=False,
        num_devices=1,
    )
    q_dram = nc.dram_tensor("q", [128, T], f32, kind="ExternalInput")
    k_dram = nc.dram_tensor("k", [128, T], bf16, kind="ExternalInput")
    kf_lo_dram = nc.dram_tensor("kf_lo", [128, T], bf16, kind="ExternalInput")
    kf_hi_dram = nc.dram_tensor("kf_hi", [128, T], bf16, kind="ExternalInput")
    qf_lo_dram = nc.dram_tensor("qf_lo", [128, NF], f32, kind="ExternalInput")
    qf_hi_dram = nc.dram_tensor("qf_hi", [128, NF], f32, kind="ExternalInput")
    w_dram = nc.dram_tensor("w", [128, 4, 12, 32], bf16, kind="ExternalInput")
    w2_dram = nc.dram_tensor("w2", [128, NPI, 32], bf16, kind="ExternalInput")
    wk_dram = nc.dram_tensor("wk", [128, 128], bf16, kind="ExternalInput")
    qs_dram = nc.dram_tensor("qs", [128, NGROUPS], f32, kind="ExternalInput")
    out_dram = nc.dram_tensor("out", [NGROUPS, 128, T], f32, kind="ExternalOutput")

    warmup = 10
    with tile.TileContext(nc) as tc:
        with (
            tc.tile_pool(name="const", bufs=1) as cpool,
            tc.tile_pool(name="ad", bufs=8) as adpool,
            tc.tile_pool(name="rl", bufs=6) as rlpool,
            tc.tile_pool(name="ft", bufs=4) as ftpool,
            tc.tile_pool(name="osb", bufs=3) as opool,
            tc.tile_pool(name="psum", bufs=4, space="PSUM") as ppool,
            tc.tile_pool(name="wpsum", bufs=1, space="PSUM") as wppool,
        ):
            q_sb = cpool.tile([128, T], f32, tag="q")
            k_sb = cpool.tile([128, T], bf16, tag="k")
            kf_lo = cpool.tile([128, T], bf16, tag="kflo")
            kf_hi = cpool.tile([128, T], bf16, tag="kfhi")
            qf_lo = cpool.tile([128, NF], f32, tag="qflo")
            qf_hi = cpool.tile([128, NF], f32, tag="qfhi")
            w_sb = cpool.tile([128, 4, 12, 32], bf16, tag="w")
            w2_sb = cpool.tile([128, NPI, 32], bf16, tag="w2")
            wk_sb = cpool.tile([128, 128], bf16, tag="wk")
            qs_sb = cpool.tile([128, NGROUPS], f32, tag="qs")

            # PE warmup: ramp the Tensor engine to full pstate on junk data
            # while the input DMAs are in flight.
            wmv = cpool.tile([128, 128], bf16, tag="wmv")
            nc.gpsimd.memset(wmv[:], 0.0)
            wps = wppool.tile([128, T], f32, tag="wps")
            for _ in range(warmup):
                nc.tensor.matmul(
                    wps[0:32, 0:128], wmv[:, 0:32], wmv[:], start=True, stop=True
                )

            # Inputs spread across DGE queues (sync/scalar/gpsimd) so the
            # copies run in parallel, ordered by first use.
            nc.sync.dma_start(wk_sb[:], wk_dram[:])
            nc.sync.dma_start(k_sb[:], k_dram[:])
            nc.sync.dma_start(kf_lo[:], kf_lo_dram[:])
            nc.sync.dma_start(kf_hi[:], kf_hi_dram[:])
            nc.sync.dma_start(w2_sb[:], w2_dram[:])
            nc.scalar.dma_start(q_sb[:], q_dram[:])
            nc.scalar.dma_start(w_sb[:, 0:2], w_dram[:, 0:2])
            nc.scalar.dma_start(w_sb[:, 2:4], w_dram[:, 2:4])
            nc.scalar.dma_start(qf_lo[:], qf_lo_dram[:])
            nc.scalar.dma_start(qf_hi[:], qf_hi_dram[:])
            nc.gpsimd.dma_start(qs_sb[:], qs_dram[:])

            def emit_block(g, c, blk, blk_pos):
                """Producers + matmuls for block c of group g into psum blk."""
                # folded pairs first: ScalarE makes both queries' relu
                # halves, Pool folds d-halves for the whole pair at once
                fts = []
                for pi in range(len(FOLD_PAIRS)):
                    m = _fold_col(g, c, pi)
                    rl_lo = rlpool.tile([128, T], bf16, tag="rl")
                    nc.scalar.activation(
                        rl_lo[:],
                        kf_lo[:],
                        mybir.ActivationFunctionType.Relu,
                        bias=qf_lo[:, m : m + 1],
                        scale=-1.0,
                    )
                    rl_hi = rlpool.tile([128, T], bf16, tag="rl")
                    nc.scalar.activation(
                        rl_hi[:],
                        kf_hi[:],
                        mybir.ActivationFunctionType.Relu,
                        bias=qf_hi[:, m : m + 1],
                        scale=-1.0,
                    )
                    ft = ftpool.tile([128, T], bf16, tag="ft")
                    nc.gpsimd.tensor_tensor(
                        ft[:], rl_lo[:], rl_hi[:], mybir.AluOpType.add
                    )
                    fts.append(ft)
                odd = (g * 4 + c) % 2 == 1
                if odd:
                    # third pair on DVE (min identity), folded on DVE
                    m = _fold_col(g, c, 2)
                    x_lo = rlpool.tile([128, T], bf16, tag="rl")
                    nc.vector.tensor_scalar_min(
                        x_lo[:], kf_lo[:], qf_lo[:, m : m + 1]
                    )
                    x_hi = rlpool.tile([128, T], bf16, tag="rl")
                    nc.vector.tensor_scalar_min(
                        x_hi[:], kf_hi[:], qf_hi[:, m : m + 1]
                    )
                    ftx = ftpool.tile([128, T], bf16, tag="ft")
                    nc.vector.tensor_tensor(
                        ftx[:], x_lo[:], x_hi[:], mybir.AluOpType.add
                    )
                    fts.append(ftx)
                for jj in range(10 if odd else 12):
                    s = 64 * g + 16 * c + jj
                    ad = adpool.tile([128, T], bf16, tag="ad")
                    nc.vector.tensor_scalar_min(
                        ad[:], k_sb[:], q_sb[:, s : s + 1]
                    )
                    nc.tensor.matmul(
                        blk,
                        w_sb[:, c, jj, :],
                        ad[:],
                        start=False,
                        stop=False,
                        tile_position=blk_pos,
                    )
                for pi, ft in enumerate(fts):
                    nc.tensor.matmul(
                        blk,
                        w2_sb[:, pi, :],
                        ft[:],
                        start=False,
                        stop=(pi == len(fts) - 1),
                        tile_position=blk_pos,
                    )

            total = NGROUPS * reps
            for gi in range(total):
                g = gi % NGROUPS
                last = gi == total - 1
                psum_t = ppool.tile([128, T], f32, tag="acc")
                # -scale * K_t correction (all 128 rows; 96 on the last
                # group, whose final block lives in the warmup psum tile so
                # the main copy+DMA can overlap its matmuls)
                nc.tensor.matmul(
                    psum_t[0:96, :] if last else psum_t[:],
                    wk_sb[:, 0:96] if last else wk_sb[:],
                    k_sb[:],
                    start=True,
                    stop=False,
                )
                if last:
                    nc.tensor.matmul(
                        wps[0:32, :],
                        wk_sb[:, 96:128],
                        k_sb[:],
                        start=True,
                        stop=False,
                        tile_position=(0, 0),
                    )
                ob = opool.tile([128, T], f32, tag="ob")
                for c in range(4):
                    blk = (
                        wps[0:32, :]
                        if (last and c == 3)
                        else psum_t[32 * c : 32 * c + 32, :]
                    )
                    emit_block(
                        g, c, blk, (0, 0) if (last and c == 3) else (0, 32 * c)
                    )
                    if last and c == 2:
                        # blocks 0-2 final: copy + bias + DMA now, overlapping
                        # block 3's matmuls
                        nc.scalar.activation(
                            ob[0:96, :],
                            psum_t[0:96, :],
                            mybir.ActivationFunctionType.Identity,
                            bias=qs_sb[0:96, g : g + 1],
                            scale=1.0,
                        )
                        nc.sync.dma_start(out_dram[g, 0:96, :], ob[0:96, :])
                # copy + per-row bias (+-scale*Q_s) on ScalarE
                if last:
                    nc.scalar.activation(
                        ob[96:128, :],
                        wps[0:32, :],
                        mybir.ActivationFunctionType.Identity,
                        bias=qs_sb[96:128, g : g + 1],
                        scale=1.0,
                    )
                    nc.sync.dma_start(out_dram[g, 96:128, :], ob[96:128, :])
                else:
                    nc.scalar.activation(
                        ob[:],
                        psum_t[:],
                        mybir.ActivationFunctionType.Identity,
                        bias=qs_sb[:, g : g + 1],
                        scale=1.0,
                    )
                    nc.sync.dma_start(out_dram[g], ob[:])

    nc.compile()
    return nc


def _host_weights():
    # Unfolded (min-identity) selector: row r = 32c + 2jj + p gets +2*scale
    # on the pair-p partition rows of the (dhalf, pair, d32) layout.
    pair_rows = np.zeros((2, 128), bool)
    for p in range(2):
        pair_rows[p, p * 32 : (p + 1) * 32] = True
        pair_rows[p, 64 + p * 32 : 64 + (p + 1) * 32] = True
    w = np.zeros((128, 4, 12, 32), np.float32)
    for c in range(4):
        for jj in range(12):
            for p in range(2):
                w[pair_rows[p], c, jj, 2 * jj + p] = 2.0 * SCALE
    # Folded (relu-identity) selector: moving tile = [foldA (pair,d32) 64;
    # foldB 64] for pair (jjA, jjB); weight -2*scale.
    w2 = np.zeros((128, NPI, 32), np.float32)
    for pi, (ja, jb) in enumerate(FOLD_PAIRS + (XTRA_PAIR,)):
        v = (2.0 if pi == 2 else -2.0) * SCALE  # min vs relu identity
        w2[0:32, pi, 2 * ja] = v
        w2[32:64, pi, 2 * ja + 1] = v
        w2[64:96, pi, 2 * jb] = v
        w2[96:128, pi, 2 * jb + 1] = v
    # K_t correction: -scale on every (pair,d) row of matching pair
    wk = np.zeros((128, 128), np.float32)
    for p in range(2):
        wk[np.ix_(pair_rows[p], np.arange(p, 128, 2))] = -SCALE
    return (
        w.astype(ml_dtypes.bfloat16),
        w2.astype(ml_dtypes.bfloat16),
        wk.astype(ml_dtypes.bfloat16),
    )


def _host_qsum(qb):
    """qb: [128, T] per-core stacked q^T in bf16, (pair,d) layout (pre-PERM).
    Returns qs [128, NGROUPS] fp32: row r = 32c + 2jj + p of group g gets
    -+scale*sum_d qb[pair p, d, s] (+ for folded/relu slots jj>=12)."""
    qsum = qb.astype(np.float64).reshape(2, 64, T).sum(axis=1)  # [pair, s]
    folded = {jj for pr in FOLD_PAIRS for jj in pr}
    qs = np.empty((128, NGROUPS), np.float64)
    for g in range(NGROUPS):
        for c in range(4):
            for jj in range(16):
                s = 64 * g + 16 * c + jj
                sign = 1.0 if jj in folded else -1.0
                for p in range(2):
                    qs[32 * c + 2 * jj + p, g] = sign * SCALE * qsum[p, s]
    return qs.astype(np.float32)


def get_module(reps=1):
    key = ("nc", reps)
    nc = _cached.get(key)
    if nc is None:
        nc = _build_module(reps)
        _cached[key] = nc
    return nc


def make_in_maps(q, k):
    """Shard full [B,T,H,D] q/k into 8 per-core input maps."""
    q = np.asarray(q, dtype=np.float32)
    k = np.asarray(k, dtype=np.float32)
    # [B, T, H, D] -> [B, H, D, T] -> [B*H, D, T]
    qt = np.ascontiguousarray(q.transpose(0, 2, 3, 1)).reshape(B * H, D, T)
    kt = np.ascontiguousarray(k.transpose(0, 2, 3, 1)).reshape(B * H, D, T)
    w, w2, wk = _host_weights()
    in_maps = []
    for core in range(NCORES):
        qc = np.ascontiguousarray(qt[2 * core : 2 * core + 2].reshape(128, T))
        kc = np.ascontiguousarray(kt[2 * core : 2 * core + 2].reshape(128, T))
        qb = qc.astype(ml_dtypes.bfloat16)
        kb = kc.astype(ml_dtypes.bfloat16)
        # PERM layout views
        qp = qb.astype(np.float32)[PERM]  # fp32 scalar source, pre-rounded
        kp = kb[PERM]
        # duplicated-row fold tiles and interleaved fold scalars
        kf_lo = np.concatenate([kp[0:64], kp[0:64]])
        kf_hi = np.concatenate([kp[64:128], kp[64:128]])
        qf_lo = np.empty((128, NF), np.float32)
        qf_hi = np.empty((128, NF), np.float32)
        for g in range(NGROUPS):
            for c in range(4):
                for pi, (ja, jb) in enumerate(FOLD_PAIRS + (XTRA_PAIR,)):
                    m = _fold_col(g, c, pi)
                    sa = 64 * g + 16 * c + ja
                    sb = 64 * g + 16 * c + jb
                    qf_lo[0:64, m] = qp[0:64, sa]
                    qf_lo[64:128, m] = qp[0:64, sb]
                    qf_hi[0:64, m] = qp[64:128, sa]
                    qf_hi[64:128, m] = qp[64:128, sb]
        in_maps.append(
            {
                "q": np.ascontiguousarray(qp),
                "k": np.ascontiguousarray(kp),
                "kf_lo": np.ascontiguousarray(kf_lo),
                "kf_hi": np.ascontiguousarray(kf_hi),
                "qf_lo": qf_lo,
                "qf_hi": qf_hi,
                "w": w,
                "w2": w2,
                "wk": wk,
                "qs": _host_qsum(qb),
            }
        )
    return in_maps


def assemble_output(core_outs):
    """core_outs: list of 8 arrays [NGROUPS, 128, T] -> full [B, T, T, H]."""
    outf = np.empty((B, T, T, H), np.float32)
    for core in range(NCORES):
        o = np.asarray(core_outs[core]).reshape(NGROUPS, 4, 16, 2, T)
        # row r = 32c + 2jj + p in group g  ->  query s = 64g + 16c + jj
        o = o.transpose(3, 0, 1, 2, 4).reshape(2, T, T)
        for p in range(2):
            pg = 2 * core + p
            b, h = divmod(pg, H)
            outf[b, :, :, h] = o[p]
    return outf


def kernel(q, k):
    from concourse.bass_utils import run_bass_kernel_spmd

    nc = get_module()
    in_maps = make_in_maps(q, k)
    res = run_bass_kernel_spmd(
        nc,
        in_maps,
        core_ids=list(range(NCORES)),
        trace=os.environ.get("BASS_L1_TRACE", "0") == "1",
    )
    _cached["last_results"] = res
    return assemble_output([r["out"] for r in res.results])


# revision 39
# speedup vs baseline: 1.2269x; 1.0193x over previous
"""L1-distance attention kernel for Trainium2 (8 NeuronCores, SPMD).

Problem: q, k: [B=2, T=512, H=8, D=64] fp32
         out[b,s,t,h] = -sum_d |q[b,s,h,d] - k[b,t,h,d]| / sqrt(D)

Sharding: 16 (b,h) pairs across 8 cores, 2 pairs per core, stacked in the
SBUF partition dim with layout (dhalf, pair, d32): partition
p = dhalf*64 + pair*32 + (d%32), dhalf = d//32. This makes the d-half fold
(below) a contiguous partition-range add.

Math: |q-k| = (q+k) - 2*min(q,k) and min(q,k) = q - relu(q-k), so with
Q_s = sum_d q[d,s], K_t = sum_d k[d,t]:
  -scale*sum_d|q-k| = 2*scale*sum_d min(k_t, q_s) - scale*K_t - scale*Q_s
                    = -2*scale*sum_d relu(q_s-k_t) - scale*K_t + scale*Q_s

All tensor data is bf16 (inputs rounded on host; the identities are exact
in bf16-value space, so only input representation error ~2^-9 and the
fold's bf16 rounding remain, far under the 2e-2 gate).

Per core, per 64-query group -> one [128, 512] fp32 PSUM tile (row
r = 32c + 2jj + p for block c, slot jj, pair p):
  - one full-width -scale*K_t matmul seeds the accumulation (start=True;
    the first five are pre-issued right after k lands to fill the PE idle
    window while the selector DMA is in flight),
  - unfolded slots (DVE/min identity): tensor_scalar_min (194ns) ->
    bf16 [128,512] tile -> one [128,32]x[128,512] bf16 selector matmul
    (213ns, weight +2*scale),
  - FOLDED slot pairs share one matmul per 2 queries: host prebuilds
    duplicated-row tiles kf_lo=[k_lo;k_lo], kf_hi=[k_hi;k_hi] and
    interleaved scalar columns qf_lo/qf_hi, so one producer op makes both
    queries' lo-half tiles (another the hi-halves), then ONE tensor_tensor
    add folds d-halves for the whole pair -> [128,512] moving tile -> ONE
    matmul covers 2 queries. Pairs (12,13),(14,15) are produced on ScalarE
    via the relu identity (weight -2*scale) and folded on Pool; on odd
    blocks pair (10,11) is additionally produced AND folded on DVE via the
    min identity (weight +2*scale). PE does 55 matmuls/group instead of
    65; the fold work lands on the Scalar/Pool/Vector engines, which sit
    at 76-89% utilization while the PE stays the 100%-busy bottleneck
    (93.7us steady-state floor, TimelineSim-verified).
  - ScalarE Identity copy folds the per-row +-scale*Q_s bias -> SBUF -> DMA.

The last group splits its final 32 rows into the warmup PSUM tile so the
main 96-row copy+DMA overlaps the final block's matmuls (shorter tail).

Host builds selector weights / Q-sum biases and unscrambles output rows.
"""

import os

import numpy as np
import ml_dtypes

os.environ.setdefault("MYCRO_LOCAL_CACHE", "1")

B, T, H, D = 2, 512, 8, 64
NCORES = 8
NGROUPS = 8  # query groups of 64 -> one PSUM tile each
SCALE = 1.0 / float(np.sqrt(np.float64(D)))  # 0.125
FOLD_PAIRS = ((12, 13), (14, 15))  # ACT-produced folded pairs, every block
XTRA_PAIR = (10, 11)  # DVE-produced pair, folded on odd blocks only
NPI = 3
NF = NGROUPS * 4 * NPI  # fold-scalar columns

# partition permutation: PERM[newp] = old row (pair*64 + d)
PERM = np.empty(128, np.int64)
for _pair in range(2):
    for _d in range(64):
        PERM[(_d // 32) * 64 + _pair * 32 + (_d % 32)] = _pair * 64 + _d

_cached = {}


def _fold_col(g, c, pi):
    return (g * 4 + c) * NPI + pi


def _build_module(reps=1):
    from concourse import bacc, tile
    import concourse.mybir as mybir

    f32 = mybir.dt.float32
    bf16 = mybir.dt.bfloat16
    nc = bacc.Bacc(
        "TRN2",
        target_bir_lowering=False,
        debug=False,
        enable_asserts=False,
        num_devices=1,
    )
    q_dram = nc.dram_tensor("q", [128, T], f32, kind="ExternalInput")
    k_dram = nc.dram_tensor("k", [128, T], bf16, kind="ExternalInput")
    kf_lo_dram = nc.dram_tensor("kf_lo", [128, T], bf16, kind="ExternalInput")
    kf_hi_dram = nc.dram_tensor("kf_hi", [128, T], bf16, kind="ExternalInput")
    qf_lo_dram = nc.dram_tensor("qf_lo", [128, NF], f32, kind="ExternalInput")
    qf_hi_dram = nc.dram_tensor("qf_hi", [128, NF], f32, kind="ExternalInput")
    w_dram = nc.dram_tensor("w", [128, 4, 12, 32], bf16, kind="ExternalInput")
    w2_dram = nc.dram_tensor("w2", [128, NPI, 32], bf16, kind="ExternalInput")
    wk_dram = nc.dram_tensor("wk", [128, 128], bf16, kind="ExternalInput")
    qs_dram = nc.dram_tensor("qs", [128, NGROUPS], f32, kind="ExternalInput")
    out_dram = nc.dram_tensor("out", [NGROUPS, 128, T], f32, kind="ExternalOutput")

    warmup = 10
    with tile.TileContext(nc) as tc:
        with (
            tc.tile_pool(name="const", bufs=1) as cpool,
            tc.tile_pool(name="ad", bufs=8) as adpool,
            tc.tile_pool(name="rl", bufs=6) as rlpool,
            tc.tile_pool(name="ft", bufs=4) as ftpool,
            tc.tile_pool(name="osb", bufs=3) as opool,
            tc.tile_pool(name="psum", bufs=5, space="PSUM") as ppool,
            tc.tile_pool(name="wpsum", bufs=1, space="PSUM") as wppool,
        ):
            q_sb = cpool.tile([128, T], f32, tag="q")
            k_sb = cpool.tile([128, T], bf16, tag="k")
            kf_lo = cpool.tile([128, T], bf16, tag="kflo")
            kf_hi = cpool.tile([128, T], bf16, tag="kfhi")
            qf_lo = cpool.tile([128, NF], f32, tag="qflo")
            qf_hi = cpool.tile([128, NF], f32, tag="qfhi")
            w_sb = cpool.tile([128, 4, 12, 32], bf16, tag="w")
            w2_sb = cpool.tile([128, NPI, 32], bf16, tag="w2")
            wk_sb = cpool.tile([128, 128], bf16, tag="wk")
            qs_sb = cpool.tile([128, NGROUPS], f32, tag="qs")

            # PE warmup: ramp the Tensor engine to full pstate on junk data
            # while the input DMAs are in flight.
            wmv = cpool.tile([128, 128], bf16, tag="wmv")
            nc.gpsimd.memset(wmv[:], 0.0)
            wps = wppool.tile([128, T], f32, tag="wps")
            for _ in range(warmup):
                nc.tensor.matmul(
                    wps[0:32, 0:128], wmv[:, 0:32], wmv[:], start=True, stop=True
                )

            # Inputs spread across DGE queues (sync/scalar/gpsimd) so the
            # copies run in parallel, ordered by first use.
            nc.sync.dma_start(wk_sb[:], wk_dram[:])
            nc.sync.dma_start(k_sb[:], k_dram[:])
            nc.sync.dma_start(kf_lo[:], kf_lo_dram[:])
            nc.sync.dma_start(kf_hi[:], kf_hi_dram[:])
            nc.sync.dma_start(w2_sb[:], w2_dram[:])
            nc.scalar.dma_start(q_sb[:], q_dram[:])
            nc.scalar.dma_start(qf_lo[:], qf_lo_dram[:])
            nc.scalar.dma_start(qf_hi[:], qf_hi_dram[:])
            nc.gpsimd.dma_start(w_sb[:, 0:2], w_dram[:, 0:2])
            nc.scalar.dma_start(w_sb[:, 2:4], w_dram[:, 2:4])
            nc.gpsimd.dma_start(qs_sb[:], qs_dram[:])

            def emit_block(g, c, blk, blk_pos):
                """Producers + matmuls for block c of group g into psum blk."""
                # folded pairs first: ScalarE makes both queries' relu
                # halves, Pool folds d-halves for the whole pair at once
                fts = []
                for pi in range(len(FOLD_PAIRS)):
                    m = _fold_col(g, c, pi)
                    rl_lo = rlpool.tile([128, T], bf16, tag="rl")
                    nc.scalar.activation(
                        rl_lo[:],
                        kf_lo[:],
                        mybir.ActivationFunctionType.Relu,
                        bias=qf_lo[:, m : m + 1],
                        scale=-1.0,
                    )
                    rl_hi = rlpool.tile([128, T], bf16, tag="rl")
                    nc.scalar.activation(
                        rl_hi[:],
                        kf_hi[:],
                        mybir.ActivationFunctionType.Relu,
                        bias=qf_hi[:, m : m + 1],
                        scale=-1.0,
                    )
                    ft = ftpool.tile([128, T], bf16, tag="ft")
                    nc.gpsimd.tensor_tensor(
                        ft[:], rl_lo[:], rl_hi[:], mybir.AluOpType.add
                    )
                    fts.append(ft)
                odd = (g * 4 + c) % 2 == 1
                if odd:
                    # third pair on DVE (min identity), folded on DVE
                    m = _fold_col(g, c, 2)
                    x_lo = rlpool.tile([128, T], bf16, tag="rl")
                    nc.vector.tensor_scalar_min(
                        x_lo[:], kf_lo[:], qf_lo[:, m : m + 1]
                    )
                    x_hi = rlpool.tile([128, T], bf16, tag="rl")
                    nc.vector.tensor_scalar_min(
                        x_hi[:], kf_hi[:], qf_hi[:, m : m + 1]
                    )
                    ftx = ftpool.tile([128, T], bf16, tag="ft")
                    nc.vector.tensor_tensor(
                        ftx[:], x_lo[:], x_hi[:], mybir.AluOpType.add
                    )
                    fts.append(ftx)
                for jj in range(10 if odd else 12):
                    s = 64 * g + 16 * c + jj
                    ad = adpool.tile([128, T], bf16, tag="ad")
                    nc.vector.tensor_scalar_min(
                        ad[:], k_sb[:], q_sb[:, s : s + 1]
                    )
                    nc.tensor.matmul(
                        blk,
                        w_sb[:, c, jj, :],
                        ad[:],
                        start=False,
                        stop=False,
                        tile_position=blk_pos,
                    )
                for pi, ft in enumerate(fts):
                    nc.tensor.matmul(
                        blk,
                        w2_sb[:, pi, :],
                        ft[:],
                        start=False,
                        stop=(pi == len(fts) - 1),
                        tile_position=blk_pos,
                    )

            total = NGROUPS * reps
            # Pre-issue the K-corrections for the first groups right after k
            # lands: they fill the PE idle window while the w selector DMA is
            # still in flight (one fewer matmul inside those groups later).
            pre_psum = []
            for gi in range(min(5, total - 1)):
                psum_p = ppool.tile([128, T], f32, tag="acc")
                nc.tensor.matmul(
                    psum_p[:], wk_sb[:], k_sb[:], start=True, stop=False
                )
                pre_psum.append(psum_p)
            for gi in range(total):
                g = gi % NGROUPS
                last = gi == total - 1
                if gi < len(pre_psum):
                    psum_t = pre_psum[gi]
                else:
                    psum_t = ppool.tile([128, T], f32, tag="acc")
                    # -scale * K_t correction (all 128 rows; 96 on the last
                    # group, whose final block lives in the warmup psum tile
                    # so the main copy+DMA can overlap its matmuls)
                    nc.tensor.matmul(
                        psum_t[0:96, :] if last else psum_t[:],
                        wk_sb[:, 0:96] if last else wk_sb[:],
                        k_sb[:],
                        start=True,
                        stop=False,
                    )
                if last:
                    nc.tensor.matmul(
                        wps[0:32, :],
                        wk_sb[:, 96:128],
                        k_sb[:],
                        start=True,
                        stop=False,
                        tile_position=(0, 0),
                    )
                ob = opool.tile([128, T], f32, tag="ob")
                for c in range(4):
                    blk = (
                        wps[0:32, :]
                        if (last and c == 3)
                        else psum_t[32 * c : 32 * c + 32, :]
                    )
                    emit_block(
                        g, c, blk, (0, 0) if (last and c == 3) else (0, 32 * c)
                    )
                    if last and c == 2:
                        # blocks 0-2 final: copy + bias + DMA now, overlapping
                        # block 3's matmuls
                        nc.scalar.activation(
                            ob[0:96, :],
                            psum_t[0:96, :],
                            mybir.ActivationFunctionType.Identity,
                            bias=qs_sb[0:96, g : g + 1],
                            scale=1.0,
                        )
                        nc.sync.dma_start(out_dram[g, 0:96, :], ob[0:96, :])
                # copy + per-row bias (+-scale*Q_s) on ScalarE
                if last:
                    nc.scalar.activation(
                        ob[96:128, :],
                        wps[0:32, :],
                        mybir.ActivationFunctionType.Identity,
                        bias=qs_sb[96:128, g : g + 1],
                        scale=1.0,
                    )
                    nc.sync.dma_start(out_dram[g, 96:128, :], ob[96:128, :])
                else:
                    nc.scalar.activation(
                        ob[:],
                        psum_t[:],
                        mybir.ActivationFunctionType.Identity,
                        bias=qs_sb[:, g : g + 1],
                        scale=1.0,
                    )
                    nc.sync.dma_start(out_dram[g], ob[:])

    nc.compile()
    return nc


def _host_weights():
    # Unfolded (min-identity) selector: row r = 32c + 2jj + p gets +2*scale
    # on the pair-p partition rows of the (dhalf, pair, d32) layout.
    pair_rows = np.zeros((2, 128), bool)
    for p in range(2):
        pair_rows[p, p * 32 : (p + 1) * 32] = True
        pair_rows[p, 64 + p * 32 : 64 + (p + 1) * 32] = True
    w = np.zeros((128, 4, 12, 32), np.float32)
    for c in range(4):
        for jj in range(12):
            for p in range(2):
                w[pair_rows[p], c, jj, 2 * jj + p] = 2.0 * SCALE
    # Folded (relu-identity) selector: moving tile = [foldA (pair,d32) 64;
    # foldB 64] for pair (jjA, jjB); weight -2*scale.
    w2 = np.zeros((128, NPI, 32), np.float32)
    for pi, (ja, jb) in enumerate(FOLD_PAIRS + (XTRA_PAIR,)):
        v = (2.0 if pi == 2 else -2.0) * SCALE  # min vs relu identity
        w2[0:32, pi, 2 * ja] = v
        w2[32:64, pi, 2 * ja + 1] = v
        w2[64:96, pi, 2 * jb] = v
        w2[96:128, pi, 2 * jb + 1] = v
    # K_t correction: -scale on every (pair,d) row of matching pair
    wk = np.zeros((128, 128), np.float32)
    for p in range(2):
        wk[np.ix_(pair_rows[p], np.arange(p, 128, 2))] = -SCALE
    return (
        w.astype(ml_dtypes.bfloat16),
        w2.astype(ml_dtypes.bfloat16),
        wk.astype(ml_dtypes.bfloat16),
    )


def _host_qsum(qb):
    """qb: [128, T] per-core stacked q^T in bf16, (pair,d) layout (pre-PERM).
    Returns qs [128, NGROUPS] fp32: row r = 32c + 2jj + p of group g gets
    -+scale*sum_d qb[pair p, d, s] (+ for folded/relu slots jj>=12)."""
    qsum = qb.astype(np.float64).reshape(2, 64, T).sum(axis=1)  # [pair, s]
    folded = {jj for pr in FOLD_PAIRS for jj in pr}
    qs = np.empty((128, NGROUPS), np.float64)
    for g in range(NGROUPS):
        for c in range(4):
            for jj in range(16):
                s = 64 * g + 16 * c + jj
                sign = 1.0 if jj in folded else -1.0
                for p in range(2):
                    qs[32 * c + 2 * jj + p, g] = sign * SCALE * qsum[p, s]
    return qs.astype(np.float32)


def get_module(reps=1):
    key = ("nc", reps)
    nc = _cached.get(key)
    if nc is None:
        nc = _build_module(reps)
        _cached[key] = nc
    return nc


def make_in_maps(q, k):
    """Shard full [B,T,H,D] q/k into 8 per-core input maps."""
    q = np.asarray(q, dtype=np.float32)
    k = np.asarray(k, dtype=np.float32)
    # [B, T, H, D] -> [B, H, D, T] -> [B*H, D, T]
    qt = np.ascontiguousarray(q.transpose(0, 2, 3, 1)).reshape(B * H, D, T)
    kt = np.ascontiguousarray(k.transpose(0, 2, 3, 1)).reshape(B * H, D, T)
    w, w2, wk = _host_weights()
    in_maps = []
    for core in range(NCORES):
        qc = np.ascontiguousarray(qt[2 * core : 2 * core + 2].reshape(128, T))
        kc = np.ascontiguousarray(kt[2 * core : 2 * core + 2].reshape(128, T))
        qb = qc.astype(ml_dtypes.bfloat16)
        kb = kc.astype(ml_dtypes.bfloat16)
        # PERM layout views
        qp = qb.astype(np.float32)[PERM]  # fp32 scalar source, pre-rounded
        kp = kb[PERM]
        # duplicated-row fold tiles and interleaved fold scalars
        kf_lo = np.concatenate([kp[0:64], kp[0:64]])
        kf_hi = np.concatenate([kp[64:128], kp[64:128]])
        qf_lo = np.empty((128, NF), np.float32)
        qf_hi = np.empty((128, NF), np.float32)
        for g in range(NGROUPS):
            for c in range(4):
                for pi, (ja, jb) in enumerate(FOLD_PAIRS + (XTRA_PAIR,)):
                    m = _fold_col(g, c, pi)
                    sa = 64 * g + 16 * c + ja
                    sb = 64 * g + 16 * c + jb
                    qf_lo[0:64, m] = qp[0:64, sa]
                    qf_lo[64:128, m] = qp[0:64, sb]
                    qf_hi[0:64, m] = qp[64:128, sa]
                    qf_hi[64:128, m] = qp[64:128, sb]
        in_maps.append(
            {
                "q": np.ascontiguousarray(qp),
                "k": np.ascontiguousarray(kp),
                "kf_lo": np.ascontiguousarray(kf_lo),
                "kf_hi": np.ascontiguousarray(kf_hi),
                "qf_lo": qf_lo,
                "qf_hi": qf_hi,
                "w": w,
                "w2": w2,
                "wk": wk,
                "qs": _host_qsum(qb),
            }
        )
    return in_maps


def assemble_output(core_outs):
    """core_outs: list of 8 arrays [NGROUPS, 128, T] -> full [B, T, T, H]."""
    outf = np.empty((B, T, T, H), np.float32)
    for core in range(NCORES):
        o = np.asarray(core_outs[core]).reshape(NGROUPS, 4, 16, 2, T)
        # row r = 32c + 2jj + p in group g  ->  query s = 64g + 16c + jj
        o = o.transpose(3, 0, 1, 2, 4).reshape(2, T, T)
        for p in range(2):
            pg = 2 * core + p
            b, h = divmod(pg, H)
            outf[b, :, :, h] = o[p]
    return outf


def kernel(q, k):
    from concourse.bass_utils import run_bass_kernel_spmd

    nc = get_module()
    in_maps = make_in_maps(q, k)
    res = run_bass_kernel_spmd(
        nc,
        in_maps,
        core_ids=list(range(NCORES)),
        trace=os.environ.get("BASS_L1_TRACE", "0") == "1",
    )
    _cached["last_results"] = res
    return assemble_output([r["out"] for r in res.results])


# revision 43
# speedup vs baseline: 1.2330x; 1.0050x over previous
"""L1-distance attention kernel for Trainium2 (8 NeuronCores, SPMD).

Problem: q, k: [B=2, T=512, H=8, D=64] fp32
         out[b,s,t,h] = -sum_d |q[b,s,h,d] - k[b,t,h,d]| / sqrt(D)

Sharding: 16 (b,h) pairs across 8 cores, 2 pairs per core, stacked in the
SBUF partition dim with layout (dhalf, pair, d32): partition
p = dhalf*64 + pair*32 + (d%32), dhalf = d//32. This makes the d-half fold
(below) a contiguous partition-range add.

Math: |q-k| = (q+k) - 2*min(q,k) and min(q,k) = q - relu(q-k), so with
Q_s = sum_d q[d,s], K_t = sum_d k[d,t]:
  -scale*sum_d|q-k| = 2*scale*sum_d min(k_t, q_s) - scale*K_t - scale*Q_s
                    = -2*scale*sum_d relu(q_s-k_t) - scale*K_t + scale*Q_s

All tensor data is bf16 (inputs rounded on host; the identities are exact
in bf16-value space, so only input representation error ~2^-9 and the
fold's bf16 rounding remain, far under the 2e-2 gate).

Per core, per 64-query group -> one [128, 512] fp32 PSUM tile (row
r = 32c + 2jj + p for block c, slot jj, pair p):
  - one full-width -scale*K_t matmul seeds the accumulation (start=True;
    the first five are pre-issued right after k lands to fill the PE idle
    window while the selector DMA is in flight),
  - unfolded slots (DVE/min identity): tensor_scalar_min (194ns) ->
    bf16 [128,512] tile -> one [128,32]x[128,512] bf16 selector matmul
    (213ns, weight +2*scale),
  - FOLDED slot pairs share one matmul per 2 queries: host prebuilds
    duplicated-row tiles kf_lo=[k_lo;k_lo], kf_hi=[k_hi;k_hi] and
    interleaved scalar columns qf_lo/qf_hi, so one producer op makes both
    queries' lo-half tiles (another the hi-halves), then ONE tensor_tensor
    add folds d-halves for the whole pair -> [128,512] moving tile -> ONE
    matmul covers 2 queries. Pairs (12,13),(14,15) are produced on ScalarE
    via the relu identity (weight -2*scale) and folded on Pool; on odd
    blocks pair (10,11) is additionally produced AND folded on DVE via the
    min identity (weight +2*scale). PE does 55 matmuls/group instead of
    65; the fold work lands on the Scalar/Pool/Vector engines, which sit
    at 76-89% utilization while the PE stays the 100%-busy bottleneck
    (93.7us steady-state floor, TimelineSim-verified).
  - ScalarE Identity copy folds the per-row +-scale*Q_s bias -> SBUF -> DMA.

The last group splits its final 32 rows into the warmup PSUM tile so the
main 96-row copy+DMA overlaps the final block's matmuls (shorter tail).

Host builds selector weights / Q-sum biases and unscrambles output rows.
"""

import os

import numpy as np
import ml_dtypes

os.environ.setdefault("MYCRO_LOCAL_CACHE", "1")

B, T, H, D = 2, 512, 8, 64
NCORES = 8
NGROUPS = 8  # query groups of 64 -> one PSUM tile each
SCALE = 1.0 / float(np.sqrt(np.float64(D)))  # 0.125
FOLD_PAIRS = ((12, 13), (14, 15))  # ACT-produced folded pairs, every block
XTRA_PAIR = (10, 11)  # DVE-produced pair, folded on odd blocks only
NPI = 3
NF = NGROUPS * 4 * NPI  # fold-scalar columns

# partition permutation: PERM[newp] = old row (pair*64 + d)
PERM = np.empty(128, np.int64)
for _pair in range(2):
    for _d in range(64):
        PERM[(_d // 32) * 64 + _pair * 32 + (_d % 32)] = _pair * 64 + _d

_cached = {}


def _fold_col(g, c, pi):
    return (g * 4 + c) * NPI + pi


def _build_module(reps=1):
    from concourse import bacc, tile
    import concourse.mybir as mybir

    f32 = mybir.dt.float32
    f16 = mybir.dt.float16
    bf16 = mybir.dt.bfloat16
    nc = bacc.Bacc(
        "TRN2",
        target_bir_lowering=False,
        debug=False,
        enable_asserts=False,
        num_devices=1,
    )
    q_dram = nc.dram_tensor("q", [128, T], f32, kind="ExternalInput")
    k_dram = nc.dram_tensor("k", [128, T], bf16, kind="ExternalInput")
    kf_lo_dram = nc.dram_tensor("kf_lo", [128, T], bf16, kind="ExternalInput")
    kf_hi_dram = nc.dram_tensor("kf_hi", [128, T], bf16, kind="ExternalInput")
    qf_lo_dram = nc.dram_tensor("qf_lo", [128, NF], f32, kind="ExternalInput")
    qf_hi_dram = nc.dram_tensor("qf_hi", [128, NF], f32, kind="ExternalInput")
    w_dram = nc.dram_tensor("w", [128, 4, 12, 32], bf16, kind="ExternalInput")
    w2_dram = nc.dram_tensor("w2", [128, NPI, 32], bf16, kind="ExternalInput")
    wk_dram = nc.dram_tensor("wk", [128, 128], bf16, kind="ExternalInput")
    qs_dram = nc.dram_tensor("qs", [128, NGROUPS], f32, kind="ExternalInput")
    out_dram = nc.dram_tensor("out", [NGROUPS, 128, T], f16, kind="ExternalOutput")

    warmup = 10
    with tile.TileContext(nc) as tc:
        with (
            tc.tile_pool(name="const", bufs=1) as cpool,
            tc.tile_pool(name="ad", bufs=8) as adpool,
            tc.tile_pool(name="rl", bufs=6) as rlpool,
            tc.tile_pool(name="ft", bufs=4) as ftpool,
            tc.tile_pool(name="osb", bufs=3) as opool,
            tc.tile_pool(name="psum", bufs=6, space="PSUM") as ppool,
            tc.tile_pool(name="wpsum", bufs=1, space="PSUM") as wppool,
        ):
            q_sb = cpool.tile([128, T], f32, tag="q")
            k_sb = cpool.tile([128, T], bf16, tag="k")
            kf_lo = cpool.tile([128, T], bf16, tag="kflo")
            kf_hi = cpool.tile([128, T], bf16, tag="kfhi")
            qf_lo = cpool.tile([128, NF], f32, tag="qflo")
            qf_hi = cpool.tile([128, NF], f32, tag="qfhi")
            w_sb = cpool.tile([128, 4, 12, 32], bf16, tag="w")
            w2_sb = cpool.tile([128, NPI, 32], bf16, tag="w2")
            wk_sb = cpool.tile([128, 128], bf16, tag="wk")
            qs_sb = cpool.tile([128, NGROUPS], f32, tag="qs")

            # PE warmup: ramp the Tensor engine to full pstate on junk data
            # while the input DMAs are in flight.
            wmv = cpool.tile([128, 128], bf16, tag="wmv")
            nc.gpsimd.memset(wmv[:], 0.0)
            wps = wppool.tile([128, T], f32, tag="wps")
            for _ in range(warmup):
                nc.tensor.matmul(
                    wps[0:32, 0:128], wmv[:, 0:32], wmv[:], start=True, stop=True
                )

            # Inputs spread across DGE queues (sync/scalar/gpsimd) so the
            # copies run in parallel, ordered by first use.
            nc.sync.dma_start(wk_sb[:], wk_dram[:])
            nc.sync.dma_start(k_sb[:], k_dram[:])
            nc.sync.dma_start(kf_lo[:], kf_lo_dram[:])
            nc.sync.dma_start(kf_hi[:], kf_hi_dram[:])
            nc.sync.dma_start(w2_sb[:], w2_dram[:])
            nc.scalar.dma_start(q_sb[:], q_dram[:])
            nc.scalar.dma_start(qf_lo[:], qf_lo_dram[:])
            nc.scalar.dma_start(qf_hi[:], qf_hi_dram[:])
            nc.gpsimd.dma_start(w_sb[:, 0:2], w_dram[:, 0:2])
            nc.scalar.dma_start(w_sb[:, 2:4], w_dram[:, 2:4])
            nc.gpsimd.dma_start(qs_sb[:], qs_dram[:])

            def emit_block(g, c, blk, blk_pos):
                """Producers + matmuls for block c of group g into psum blk."""
                # folded pairs first: ScalarE makes both queries' relu
                # halves, Pool folds d-halves for the whole pair at once
                fts = []
                for pi in range(len(FOLD_PAIRS)):
                    m = _fold_col(g, c, pi)
                    rl_lo = rlpool.tile([128, T], bf16, tag="rl")
                    nc.scalar.activation(
                        rl_lo[:],
                        kf_lo[:],
                        mybir.ActivationFunctionType.Relu,
                        bias=qf_lo[:, m : m + 1],
                        scale=-1.0,
                    )
                    rl_hi = rlpool.tile([128, T], bf16, tag="rl")
                    nc.scalar.activation(
                        rl_hi[:],
                        kf_hi[:],
                        mybir.ActivationFunctionType.Relu,
                        bias=qf_hi[:, m : m + 1],
                        scale=-1.0,
                    )
                    ft = ftpool.tile([128, T], bf16, tag="ft")
                    nc.gpsimd.tensor_tensor(
                        ft[:], rl_lo[:], rl_hi[:], mybir.AluOpType.add
                    )
                    fts.append(ft)
                odd = (g * 4 + c) % 2 == 1
                if odd:
                    # third pair on DVE (min identity), folded on DVE
                    m = _fold_col(g, c, 2)
                    x_lo = rlpool.tile([128, T], bf16, tag="rl")
                    nc.vector.tensor_scalar_min(
                        x_lo[:], kf_lo[:], qf_lo[:, m : m + 1]
                    )
                    x_hi = rlpool.tile([128, T], bf16, tag="rl")
                    nc.vector.tensor_scalar_min(
                        x_hi[:], kf_hi[:], qf_hi[:, m : m + 1]
                    )
                    ftx = ftpool.tile([128, T], bf16, tag="ft")
                    nc.vector.tensor_tensor(
                        ftx[:], x_lo[:], x_hi[:], mybir.AluOpType.add
                    )
                    fts.append(ftx)
                for jj in range(10 if odd else 12):
                    s = 64 * g + 16 * c + jj
                    ad = adpool.tile([128, T], bf16, tag="ad")
                    nc.vector.tensor_scalar_min(
                        ad[:], k_sb[:], q_sb[:, s : s + 1]
                    )
                    nc.tensor.matmul(
                        blk,
                        w_sb[:, c, jj, :],
                        ad[:],
                        start=False,
                        stop=False,
                        tile_position=blk_pos,
                    )
                for pi, ft in enumerate(fts):
                    nc.tensor.matmul(
                        blk,
                        w2_sb[:, pi, :],
                        ft[:],
                        start=False,
                        stop=(pi == len(fts) - 1),
                        tile_position=blk_pos,
                    )

            total = NGROUPS * reps
            # Pre-issue the K-corrections for the first groups right after k
            # lands: they fill the PE idle window while the w selector DMA is
            # still in flight (one fewer matmul inside those groups later).
            pre_psum = []
            for gi in range(min(6, total - 1)):
                psum_p = ppool.tile([128, T], f32, tag="acc")
                nc.tensor.matmul(
                    psum_p[:], wk_sb[:], k_sb[:], start=True, stop=False
                )
                pre_psum.append(psum_p)
            for gi in range(total):
                g = gi % NGROUPS
                last = gi == total - 1
                if gi < len(pre_psum):
                    psum_t = pre_psum[gi]
                else:
                    psum_t = ppool.tile([128, T], f32, tag="acc")
                    # -scale * K_t correction (all 128 rows; 96 on the last
                    # group, whose final block lives in the warmup psum tile
                    # so the main copy+DMA can overlap its matmuls)
                    nc.tensor.matmul(
                        psum_t[0:96, :] if last else psum_t[:],
                        wk_sb[:, 0:96] if last else wk_sb[:],
                        k_sb[:],
                        start=True,
                        stop=False,
                    )
                if last:
                    nc.tensor.matmul(
                        wps[0:32, :],
                        wk_sb[:, 96:128],
                        k_sb[:],
                        start=True,
                        stop=False,
                        tile_position=(0, 0),
                    )
                ob = opool.tile([128, T], f16, tag="ob")
                for c in range(4):
                    blk = (
                        wps[0:32, :]
                        if (last and c == 3)
                        else psum_t[32 * c : 32 * c + 32, :]
                    )
                    emit_block(
                        g, c, blk, (0, 0) if (last and c == 3) else (0, 32 * c)
                    )
                    if last and c == 2:
                        # blocks 0-2 final: copy + bias + DMA now, overlapping
                        # block 3's matmuls
                        nc.scalar.activation(
                            ob[0:96, :],
                            psum_t[0:96, :],
                            mybir.ActivationFunctionType.Identity,
                            bias=qs_sb[0:96, g : g + 1],
                            scale=1.0,
                        )
                        nc.sync.dma_start(out_dram[g, 0:96, :], ob[0:96, :])
                # copy + per-row bias (+-scale*Q_s) on ScalarE
                if last:
                    nc.scalar.activation(
                        ob[96:128, :],
                        wps[0:32, :],
                        mybir.ActivationFunctionType.Identity,
                        bias=qs_sb[96:128, g : g + 1],
                        scale=1.0,
                    )
                    nc.sync.dma_start(out_dram[g, 96:128, :], ob[96:128, :])
                else:
                    nc.scalar.activation(
                        ob[:],
                        psum_t[:],
                        mybir.ActivationFunctionType.Identity,
                        bias=qs_sb[:, g : g + 1],
                        scale=1.0,
                    )
                    nc.sync.dma_start(out_dram[g], ob[:])

    nc.compile()
    return nc


def _host_weights():
    # Unfolded (min-identity) selector: row r = 32c + 2jj + p gets +2*scale
    # on the pair-p partition rows of the (dhalf, pair, d32) layout.
    pair_rows = np.zeros((2, 128), bool)
    for p in range(2):
        pair_rows[p, p * 32 : (p + 1) * 32] = True
        pair_rows[p, 64 + p * 32 : 64 + (p + 1) * 32] = True
    w = np.zeros((128, 4, 12, 32), np.float32)
    for c in range(4):
        for jj in range(12):
            for p in range(2):
                w[pair_rows[p], c, jj, 2 * jj + p] = 2.0 * SCALE
    # Folded (relu-identity) selector: moving tile = [foldA (pair,d32) 64;
    # foldB 64] for pair (jjA, jjB); weight -2*scale.
    w2 = np.zeros((128, NPI, 32), np.float32)
    for pi, (ja, jb) in enumerate(FOLD_PAIRS + (XTRA_PAIR,)):
        v = (2.0 if pi == 2 else -2.0) * SCALE  # min vs relu identity
        w2[0:32, pi, 2 * ja] = v
        w2[32:64, pi, 2 * ja + 1] = v
        w2[64:96, pi, 2 * jb] = v
        w2[96:128, pi, 2 * jb + 1] = v
    # K_t correction: -scale on every (pair,d) row of matching pair
    wk = np.zeros((128, 128), np.float32)
    for p in range(2):
        wk[np.ix_(pair_rows[p], np.arange(p, 128, 2))] = -SCALE
    return (
        w.astype(ml_dtypes.bfloat16),
        w2.astype(ml_dtypes.bfloat16),
        wk.astype(ml_dtypes.bfloat16),
    )


def _host_qsum(qb):
    """qb: [128, T] per-core stacked q^T in bf16, (pair,d) layout (pre-PERM).
    Returns qs [128, NGROUPS] fp32: row r = 32c + 2jj + p of group g gets
    -+scale*sum_d qb[pair p, d, s] (+ for folded/relu slots jj>=12)."""
    qsum = qb.astype(np.float64).reshape(2, 64, T).sum(axis=1)  # [pair, s]
    folded = {jj for pr in FOLD_PAIRS for jj in pr}
    qs = np.empty((128, NGROUPS), np.float64)
    for g in range(NGROUPS):
        for c in range(4):
            for jj in range(16):
                s = 64 * g + 16 * c + jj
                sign = 1.0 if jj in folded else -1.0
                for p in range(2):
                    qs[32 * c + 2 * jj + p, g] = sign * SCALE * qsum[p, s]
    return qs.astype(np.float32)


def get_module(reps=1):
    key = ("nc", reps)
    nc = _cached.get(key)
    if nc is None:
        nc = _build_module(reps)
        _cached[key] = nc
    return nc


def make_in_maps(q, k):
    """Shard full [B,T,H,D] q/k into 8 per-core input maps."""
    q = np.asarray(q, dtype=np.float32)
    k = np.asarray(k, dtype=np.float32)
    # [B, T, H, D] -> [B, H, D, T] -> [B*H, D, T]
    qt = np.ascontiguousarray(q.transpose(0, 2, 3, 1)).reshape(B * H, D, T)
    kt = np.ascontiguousarray(k.transpose(0, 2, 3, 1)).reshape(B * H, D, T)
    w, w2, wk = _host_weights()
    in_maps = []
    for core in range(NCORES):
        qc = np.ascontiguousarray(qt[2 * core : 2 * core + 2].reshape(128, T))
        kc = np.ascontiguousarray(kt[2 * core : 2 * core + 2].reshape(128, T))
        qb = qc.astype(ml_dtypes.bfloat16)
        kb = kc.astype(ml_dtypes.bfloat16)
        # PERM layout views
        qp = qb.astype(np.float32)[PERM]  # fp32 scalar source, pre-rounded
        kp = kb[PERM]
        # duplicated-row fold tiles and interleaved fold scalars
        kf_lo = np.concatenate([kp[0:64], kp[0:64]])
        kf_hi = np.concatenate([kp[64:128], kp[64:128]])
        qf_lo = np.empty((128, NF), np.float32)
        qf_hi = np.empty((128, NF), np.float32)
        for g in range(NGROUPS):
            for c in range(4):
                for pi, (ja, jb) in enumerate(FOLD_PAIRS + (XTRA_PAIR,)):
                    m = _fold_col(g, c, pi)
                    sa = 64 * g + 16 * c + ja
                    sb = 64 * g + 16 * c + jb
                    qf_lo[0:64, m] = qp[0:64, sa]
                    qf_lo[64:128, m] = qp[0:64, sb]
                    qf_hi[0:64, m] = qp[64:128, sa]
                    qf_hi[64:128, m] = qp[64:128, sb]
        in_maps.append(
            {
                "q": np.ascontiguousarray(qp),
                "k": np.ascontiguousarray(kp),
                "kf_lo": np.ascontiguousarray(kf_lo),
                "kf_hi": np.ascontiguousarray(kf_hi),
                "qf_lo": qf_lo,
                "qf_hi": qf_hi,
                "w": w,
                "w2": w2,
                "wk": wk,
                "qs": _host_qsum(qb),
            }
        )
    return in_maps


def assemble_output(core_outs):
    """core_outs: list of 8 arrays [NGROUPS, 128, T] -> full [B, T, T, H]."""
    outf = np.empty((B, T, T, H), np.float32)
    for core in range(NCORES):
        o = np.asarray(core_outs[core]).astype(np.float32)
        o = o.reshape(NGROUPS, 4, 16, 2, T)
        # row r = 32c + 2jj + p in group g  ->  query s = 64g + 16c + jj
        o = o.transpose(3, 0, 1, 2, 4).reshape(2, T, T)
        for p in range(2):
            pg = 2 * core + p
            b, h = divmod(pg, H)
            outf[b, :, :, h] = o[p]
    return outf


def kernel(q, k):
    from concourse.bass_utils import run_bass_kernel_spmd

    nc = get_module()
    in_maps = make_in_maps(q, k)
    res = run_bass_kernel_spmd(
        nc,
        in_maps,
        core_ids=list(range(NCORES)),
        trace=os.environ.get("BASS_L1_TRACE", "0") == "1",
    )
    _cached["last_results"] = res
    return assemble_output([r["out"] for r in res.results])


# revision 49
# speedup vs baseline: 1.2408x; 1.0063x over previous
"""L1-distance attention kernel for Trainium2 (8 NeuronCores, SPMD).

Problem: q, k: [B=2, T=512, H=8, D=64] fp32
         out[b,s,t,h] = -sum_d |q[b,s,h,d] - k[b,t,h,d]| / sqrt(D)

Sharding: 16 (b,h) pairs across 8 cores, 2 pairs per core, stacked in the
SBUF partition dim with layout (dhalf, pair, d32): partition
p = dhalf*64 + pair*32 + (d%32), dhalf = d//32. This makes the d-half fold
(below) a contiguous partition-range add.

Math: |q-k| = (q+k) - 2*min(q,k) and min(q,k) = q - relu(q-k), so with
Q_s = sum_d q[d,s], K_t = sum_d k[d,t]:
  -scale*sum_d|q-k| = 2*scale*sum_d min(k_t, q_s) - scale*K_t - scale*Q_s
                    = -2*scale*sum_d relu(q_s-k_t) - scale*K_t + scale*Q_s

All tensor data is bf16 (inputs rounded on host; the identities are exact
in bf16-value space, so only input representation error ~2^-9 and the
fold's bf16 rounding remain, far under the 2e-2 gate).

Per core, per 64-query group -> one [128, 512] fp32 PSUM tile (row
r = 32c + 2jj + p for block c, slot jj, pair p):
  - one full-width -scale*K_t matmul seeds the accumulation (start=True;
    the first seven are pre-issued right after k lands to fill the PE idle
    window while the selector DMA is in flight),
  - unfolded slots (DVE/min identity): tensor_scalar_min (194ns) ->
    bf16 [128,512] tile -> one [128,32]x[128,512] bf16 selector matmul
    (213ns, weight +2*scale),
  - FOLDED slot pairs share one matmul per 2 queries: host prebuilds
    duplicated-row tiles kf_lo=[k_lo;k_lo], kf_hi=[k_hi;k_hi] and
    interleaved scalar columns qf_lo/qf_hi, so one producer op makes both
    queries' lo-half tiles (another the hi-halves), then ONE tensor_tensor
    add folds d-halves for the whole pair -> [128,512] moving tile -> ONE
    matmul covers 2 queries. Pairs (12,13),(14,15) are produced on ScalarE
    via the relu identity (weight -2*scale) and folded on Pool; on odd
    blocks pair (10,11) is additionally produced AND folded on DVE via the
    min identity (weight +2*scale). PE does 55 matmuls/group instead of
    65; the fold work lands on the Scalar/Pool/Vector engines, which sit
    at 76-89% utilization while the PE stays the 100%-busy bottleneck
    (93.7us steady-state floor, TimelineSim-verified).
  - ScalarE Identity copy folds the per-row +-scale*Q_s bias -> fp16
    SBUF tile (halves the output DMA; fp16 step ~0.008 at |out|~9 is
    negligible) -> DMA out; host converts back to fp32.

The last group splits its final 32 rows into the warmup PSUM tile so the
main 96-row copy+DMA overlaps the final block's matmuls (shorter tail).

Host builds selector weights / Q-sum biases and unscrambles output rows.
"""

import os

import numpy as np
import ml_dtypes

os.environ.setdefault("MYCRO_LOCAL_CACHE", "1")

B, T, H, D = 2, 512, 8, 64
NCORES = 8
NGROUPS = 8  # query groups of 64 -> one PSUM tile each
SCALE = 1.0 / float(np.sqrt(np.float64(D)))  # 0.125
FOLD_PAIRS = ((12, 13), (14, 15))  # ACT-produced folded pairs, every block
XTRA_PAIR = (10, 11)  # DVE-produced pair, folded on odd blocks only
NPI = 3
NF = NGROUPS * 4 * NPI  # fold-scalar columns

# partition permutation: PERM[newp] = old row (pair*64 + d)
PERM = np.empty(128, np.int64)
for _pair in range(2):
    for _d in range(64):
        PERM[(_d // 32) * 64 + _pair * 32 + (_d % 32)] = _pair * 64 + _d

_cached = {}


def _fold_col(g, c, pi):
    return (g * 4 + c) * NPI + pi


def _build_module(reps=1):
    from concourse import bacc, tile
    import concourse.mybir as mybir

    f32 = mybir.dt.float32
    f16 = mybir.dt.float16
    bf16 = mybir.dt.bfloat16
    nc = bacc.Bacc(
        "TRN2",
        target_bir_lowering=False,
        debug=False,
        enable_asserts=False,
        num_devices=1,
    )
    q_dram = nc.dram_tensor("q", [128, T], f32, kind="ExternalInput")
    k_dram = nc.dram_tensor("k", [128, T + 128], bf16, kind="ExternalInput")
    kf_lo_dram = nc.dram_tensor("kf_lo", [128, T], bf16, kind="ExternalInput")
    kf_hi_dram = nc.dram_tensor("kf_hi", [128, T], bf16, kind="ExternalInput")
    qf_lo_dram = nc.dram_tensor("qf_lo", [128, NF], f32, kind="ExternalInput")
    qf_hi_dram = nc.dram_tensor("qf_hi", [128, NF], f32, kind="ExternalInput")
    w_dram = nc.dram_tensor("w", [128, 4, 12, 32], bf16, kind="ExternalInput")
    w2_dram = nc.dram_tensor("w2", [128, NPI, 32], bf16, kind="ExternalInput")
    qs_dram = nc.dram_tensor("qs", [128, NGROUPS], f32, kind="ExternalInput")
    out_dram = nc.dram_tensor("out", [NGROUPS, 128, T], f16, kind="ExternalOutput")

    warmup = 10
    with tile.TileContext(nc) as tc:
        with (
            tc.tile_pool(name="const", bufs=1) as cpool,
            tc.tile_pool(name="ad", bufs=8) as adpool,
            tc.tile_pool(name="rl", bufs=6) as rlpool,
            tc.tile_pool(name="ft", bufs=4) as ftpool,
            tc.tile_pool(name="osb", bufs=3) as opool,
            tc.tile_pool(name="psum", bufs=6, space="PSUM") as ppool,
            tc.tile_pool(name="wpsum", bufs=1, space="PSUM") as wppool,
        ):
            q_sb = cpool.tile([128, T], f32, tag="q")
            k_sb = cpool.tile([128, T + 128], bf16, tag="k")
            kf_lo = cpool.tile([128, T], bf16, tag="kflo")
            kf_hi = cpool.tile([128, T], bf16, tag="kfhi")
            qf_lo = cpool.tile([128, NF], f32, tag="qflo")
            qf_hi = cpool.tile([128, NF], f32, tag="qfhi")
            w_sb = cpool.tile([128, 4, 12, 32], bf16, tag="w")
            w2_sb = cpool.tile([128, NPI, 32], bf16, tag="w2")
            qs_sb = cpool.tile([128, NGROUPS], f32, tag="qs")

            # PE warmup: ramp the Tensor engine to full pstate on junk data
            # while the input DMAs are in flight.
            wmv = cpool.tile([128, 128], bf16, tag="wmv")
            nc.gpsimd.memset(wmv[:], 0.0)
            wps = wppool.tile([128, T], f32, tag="wps")
            for _ in range(warmup):
                nc.tensor.matmul(
                    wps[0:32, 0:128], wmv[:, 0:32], wmv[:], start=True, stop=True
                )

            # Inputs spread across DGE queues (sync/scalar/gpsimd) so the
            # copies run in parallel, ordered by first use.
            nc.sync.dma_start(k_sb[:], k_dram[:])
            nc.sync.dma_start(kf_lo[:], kf_lo_dram[:])
            nc.sync.dma_start(kf_hi[:], kf_hi_dram[:])
            nc.sync.dma_start(w2_sb[:], w2_dram[:])
            nc.scalar.dma_start(q_sb[:], q_dram[:])
            nc.scalar.dma_start(qf_lo[:], qf_lo_dram[:])
            nc.scalar.dma_start(qf_hi[:], qf_hi_dram[:])
            nc.gpsimd.dma_start(w_sb[:, 0:2], w_dram[:, 0:2])
            nc.scalar.dma_start(w_sb[:, 2:4], w_dram[:, 2:4])
            nc.gpsimd.dma_start(qs_sb[:], qs_dram[:])

            def emit_producers(g, c):
                # folded pairs first: ScalarE makes both queries' relu
                # halves, Pool folds d-halves for the whole pair at once
                fts = []
                for pi in range(len(FOLD_PAIRS)):
                    m = _fold_col(g, c, pi)
                    rl_lo = rlpool.tile([128, T], bf16, tag="rl")
                    nc.scalar.activation(
                        rl_lo[:],
                        kf_lo[:],
                        mybir.ActivationFunctionType.Relu,
                        bias=qf_lo[:, m : m + 1],
                        scale=-1.0,
                    )
                    rl_hi = rlpool.tile([128, T], bf16, tag="rl")
                    nc.scalar.activation(
                        rl_hi[:],
                        kf_hi[:],
                        mybir.ActivationFunctionType.Relu,
                        bias=qf_hi[:, m : m + 1],
                        scale=-1.0,
                    )
                    ft = ftpool.tile([128, T], bf16, tag="ft")
                    nc.gpsimd.tensor_tensor(
                        ft[:], rl_lo[:], rl_hi[:], mybir.AluOpType.add
                    )
                    fts.append(ft)
                odd = (g * 4 + c) % 2 == 1
                if odd:
                    # third pair on DVE (min identity), folded on DVE
                    m = _fold_col(g, c, 2)
                    x_lo = rlpool.tile([128, T], bf16, tag="rl")
                    nc.vector.tensor_scalar_min(
                        x_lo[:], kf_lo[:], qf_lo[:, m : m + 1]
                    )
                    x_hi = rlpool.tile([128, T], bf16, tag="rl")
                    nc.vector.tensor_scalar_min(
                        x_hi[:], kf_hi[:], qf_hi[:, m : m + 1]
                    )
                    ftx = ftpool.tile([128, T], bf16, tag="ft")
                    nc.vector.tensor_tensor(
                        ftx[:], x_lo[:], x_hi[:], mybir.AluOpType.add
                    )
                    fts.append(ftx)
                return fts

            def emit_block(g, c, blk, blk_pos, fts=None):
                """Producers + matmuls for block c of group g into psum blk."""
                if fts is None:
                    fts = emit_producers(g, c)
                odd = (g * 4 + c) % 2 == 1
                for jj in range(10 if odd else 12):
                    s = 64 * g + 16 * c + jj
                    ad = adpool.tile([128, T], bf16, tag="ad")
                    nc.vector.tensor_scalar_min(
                        ad[:], k_sb[:, 0:T], q_sb[:, s : s + 1]
                    )
                    nc.tensor.matmul(
                        blk,
                        w_sb[:, c, jj, :],
                        ad[:],
                        start=False,
                        stop=False,
                        tile_position=blk_pos,
                    )
                for pi, ft in enumerate(fts):
                    nc.tensor.matmul(
                        blk,
                        w2_sb[:, pi, :],
                        ft[:],
                        start=False,
                        stop=(pi == len(fts) - 1),
                        tile_position=blk_pos,
                    )

            total = NGROUPS * reps
            # Prefetch all of group 0's fold producers so the ACT/Pool/DVE
            # chains are already full when the PE reaches the first blocks.
            g0_fts = [emit_producers(0, c) for c in range(4)]

            # Pre-issue the K-corrections for the first groups right after k
            # lands: they fill the PE idle window while the w selector DMA is
            # still in flight (one fewer matmul inside those groups later).
            pre_psum = []
            for gi in range(min(6, total - 1)):
                psum_p = ppool.tile([128, T], f32, tag="acc")
                nc.tensor.matmul(
                    psum_p[:],
                    k_sb[:, T : T + 128],
                    k_sb[:, 0:T],
                    start=True,
                    stop=False,
                )
                pre_psum.append(psum_p)
            for gi in range(total):
                g = gi % NGROUPS
                last = gi == total - 1
                if gi < len(pre_psum):
                    psum_t = pre_psum[gi]
                else:
                    psum_t = ppool.tile([128, T], f32, tag="acc")
                    # -scale * K_t correction (all 128 rows; 96 on the last
                    # group, whose final block lives in the warmup psum tile
                    # so the main copy+DMA can overlap its matmuls)
                    nc.tensor.matmul(
                        psum_t[0:96, :] if last else psum_t[:],
                        k_sb[:, T : T + 96] if last else k_sb[:, T : T + 128],
                        k_sb[:, 0:T],
                        start=True,
                        stop=False,
                    )
                if last:
                    nc.tensor.matmul(
                        wps[0:32, :],
                        k_sb[:, T + 96 : T + 128],
                        k_sb[:, 0:T],
                        start=True,
                        stop=False,
                        tile_position=(0, 0),
                    )
                ob = opool.tile([128, T], f16, tag="ob")
                for c in range(4):
                    blk = (
                        wps[0:32, :]
                        if (last and c == 3)
                        else psum_t[32 * c : 32 * c + 32, :]
                    )
                    emit_block(
                        g,
                        c,
                        blk,
                        (0, 0) if (last and c == 3) else (0, 32 * c),
                        fts=g0_fts[c] if gi == 0 else None,
                    )
                    if last and c == 2:
                        # blocks 0-2 final: copy + bias + DMA now, overlapping
                        # block 3's matmuls
                        nc.scalar.activation(
                            ob[0:96, :],
                            psum_t[0:96, :],
                            mybir.ActivationFunctionType.Identity,
                            bias=qs_sb[0:96, g : g + 1],
                            scale=1.0,
                        )
                        nc.sync.dma_start(out_dram[g, 0:96, :], ob[0:96, :])
                # copy + per-row bias (+-scale*Q_s) on ScalarE
                if last:
                    # final copy on VectorE: ScalarE is still draining block
                    # 3's relu halves + the 96-row copy, DVE is idle here
                    nc.vector.tensor_scalar_add(
                        ob[96:128, :],
                        wps[0:32, :],
                        qs_sb[96:128, g : g + 1],
                    )
                    nc.sync.dma_start(out_dram[g, 96:128, :], ob[96:128, :])
                else:
                    nc.scalar.activation(
                        ob[:],
                        psum_t[:],
                        mybir.ActivationFunctionType.Identity,
                        bias=qs_sb[:, g : g + 1],
                        scale=1.0,
                    )
                    nc.sync.dma_start(out_dram[g], ob[:])

    nc.compile()
    return nc


def _host_weights():
    # Unfolded (min-identity) selector: row r = 32c + 2jj + p gets +2*scale
    # on the pair-p partition rows of the (dhalf, pair, d32) layout.
    pair_rows = np.zeros((2, 128), bool)
    for p in range(2):
        pair_rows[p, p * 32 : (p + 1) * 32] = True
        pair_rows[p, 64 + p * 32 : 64 + (p + 1) * 32] = True
    w = np.zeros((128, 4, 12, 32), np.float32)
    for c in range(4):
        for jj in range(12):
            for p in range(2):
                w[pair_rows[p], c, jj, 2 * jj + p] = 2.0 * SCALE
    # Folded (relu-identity) selector: moving tile = [foldA (pair,d32) 64;
    # foldB 64] for pair (jjA, jjB); weight -2*scale.
    w2 = np.zeros((128, NPI, 32), np.float32)
    for pi, (ja, jb) in enumerate(FOLD_PAIRS + (XTRA_PAIR,)):
        v = (2.0 if pi == 2 else -2.0) * SCALE  # min vs relu identity
        w2[0:32, pi, 2 * ja] = v
        w2[32:64, pi, 2 * ja + 1] = v
        w2[64:96, pi, 2 * jb] = v
        w2[96:128, pi, 2 * jb + 1] = v
    # K_t correction: -scale on every (pair,d) row of matching pair
    wk = np.zeros((128, 128), np.float32)
    for p in range(2):
        wk[np.ix_(pair_rows[p], np.arange(p, 128, 2))] = -SCALE
    return (
        w.astype(ml_dtypes.bfloat16),
        w2.astype(ml_dtypes.bfloat16),
        wk.astype(ml_dtypes.bfloat16),
    )


def _host_qsum(qb):
    """qb: [128, T] per-core stacked q^T in bf16, (pair,d) layout (pre-PERM).
    Returns qs [128, NGROUPS] fp32: row r = 32c + 2jj + p of group g gets
    -+scale*sum_d qb[pair p, d, s] (+ for folded/relu slots jj>=12)."""
    qsum = qb.astype(np.float64).reshape(2, 64, T).sum(axis=1)  # [pair, s]
    folded = {jj for pr in FOLD_PAIRS for jj in pr}
    qs = np.empty((128, NGROUPS), np.float64)
    for g in range(NGROUPS):
        for c in range(4):
            for jj in range(16):
                s = 64 * g + 16 * c + jj
                sign = 1.0 if jj in folded else -1.0
                for p in range(2):
                    qs[32 * c + 2 * jj + p, g] = sign * SCALE * qsum[p, s]
    return qs.astype(np.float32)


def get_module(reps=1):
    key = ("nc", reps)
    nc = _cached.get(key)
    if nc is None:
        nc = _build_module(reps)
        _cached[key] = nc
    return nc


def make_in_maps(q, k):
    """Shard full [B,T,H,D] q/k into 8 per-core input maps."""
    q = np.asarray(q, dtype=np.float32)
    k = np.asarray(k, dtype=np.float32)
    # [B, T, H, D] -> [B, H, D, T] -> [B*H, D, T]
    qt = np.ascontiguousarray(q.transpose(0, 2, 3, 1)).reshape(B * H, D, T)
    kt = np.ascontiguousarray(k.transpose(0, 2, 3, 1)).reshape(B * H, D, T)
    w, w2, wk = _host_weights()
    in_maps = []
    for core in range(NCORES):
        qc = np.ascontiguousarray(qt[2 * core : 2 * core + 2].reshape(128, T))
        kc = np.ascontiguousarray(kt[2 * core : 2 * core + 2].reshape(128, T))
        qb = qc.astype(ml_dtypes.bfloat16)
        kb = kc.astype(ml_dtypes.bfloat16)
        # PERM layout views
        qp = qb.astype(np.float32)[PERM]  # fp32 scalar source, pre-rounded
        kp = kb[PERM]
        # duplicated-row fold tiles and interleaved fold scalars
        kf_lo = np.concatenate([kp[0:64], kp[0:64]])
        kf_hi = np.concatenate([kp[64:128], kp[64:128]])
        qf_lo = np.empty((128, NF), np.float32)
        qf_hi = np.empty((128, NF), np.float32)
        for g in range(NGROUPS):
            for c in range(4):
                for pi, (ja, jb) in enumerate(FOLD_PAIRS + (XTRA_PAIR,)):
                    m = _fold_col(g, c, pi)
                    sa = 64 * g + 16 * c + ja
                    sb = 64 * g + 16 * c + jb
                    qf_lo[0:64, m] = qp[0:64, sa]
                    qf_lo[64:128, m] = qp[0:64, sb]
                    qf_hi[0:64, m] = qp[64:128, sa]
                    qf_hi[64:128, m] = qp[64:128, sb]
        in_maps.append(
            {
                "q": np.ascontiguousarray(qp),
                "k": np.ascontiguousarray(
                    np.concatenate([kp, wk], axis=1)
                ),
                "kf_lo": np.ascontiguousarray(kf_lo),
                "kf_hi": np.ascontiguousarray(kf_hi),
                "qf_lo": qf_lo,
                "qf_hi": qf_hi,
                "w": w,
                "w2": w2,
                "wk": wk,
                "qs": _host_qsum(qb),
            }
        )
    return in_maps


def assemble_output(core_outs):
    """core_outs: list of 8 arrays [NGROUPS, 128, T] -> full [B, T, T, H]."""
    outf = np.empty((B, T, T, H), np.float32)
    for core in range(NCORES):
        o = np.asarray(core_outs[core]).astype(np.float32)
        o = o.reshape(NGROUPS, 4, 16, 2, T)
        # row r = 32c + 2jj + p in group g  ->  query s = 64g + 16c + jj
        o = o.transpose(3, 0, 1, 2, 4).reshape(2, T, T)
        for p in range(2):
            pg = 2 * core + p
            b, h = divmod(pg, H)
            outf[b, :, :, h] = o[p]
    return outf


def kernel(q, k):
    from concourse.bass_utils import run_bass_kernel_spmd

    nc = get_module()
    in_maps = make_in_maps(q, k)
    res = run_bass_kernel_spmd(
        nc,
        in_maps,
        core_ids=list(range(NCORES)),
        trace=os.environ.get("BASS_L1_TRACE", "0") == "1",
    )
    _cached["last_results"] = res
    return assemble_output([r["out"] for r in res.results])
